# revision 13
# baseline (speedup 1.0000x reference)
"""Trainium2 Bass kernel for a 4-layer transformer encoder.

Sharding: 8-way data-parallel over tokens. Cores 0-3 handle batch element 0,
cores 4-7 batch element 1; each core owns a 512-token slice of its batch
element's 2048-token sequence. Attention needs full-sequence K/V, so each
layer AllGathers the core's K/V slice within its 4-core group — split into
two token-half collectives so the wire time overlaps KV/Q compute and the
gathered loads pipeline.

On-chip layout: activations are feature-major ([d, tokens]); LayerNorm
statistics are computed with ones-vector matmuls (partition-axis reduction),
softmax runs on transposed scores ([keys, queries]) so the denominator comes
from an appended ones-column in V and no transposes are needed anywhere.
Attention processes heads in pairs: the two 64-row score matmuls of a pair
run concurrently in disjoint PE row-groups (tile_position packing), which
keeps the full 128x128 array active. Matmuls run in bf16 with fp32 PSUM
accumulation; the residual stream stays fp32 in SBUF.
"""
import numpy as np
import ml_dtypes

import concourse.bass as bass
import concourse.bacc as bacc
import concourse.tile as tile
import concourse.mybir as mybir
from concourse.bass_utils import run_bass_kernel_spmd

dt = mybir.dt
AF = mybir.ActivationFunctionType
OP = mybir.AluOpType
BF16 = ml_dtypes.bfloat16

DIM = 1024
DEPTH = 4
HEADS = 16
DH = 64
INNER = 1024
FF = 4096
EPS = 1e-5
SEQ = 2048
BATCH = 2
N_CORES = 8
GROUP = 4                 # cores per batch element
TPC = SEQ // GROUP        # tokens per core = 512
HT = TPC // 2             # tokens per AG half = 256
DC = DIM // 128           # 8 dmodel chunks
KC = SEQ // 128           # 16 key chunks
FC = FF // 128            # 32 ff chunks
VW = HEADS * (DH + 1)     # v tile row width per key chunk (ones col appended)
KV_H = DIM * HT           # elements in one k (or v) half-slice
WSL = 256                 # weight slice column width

_CACHE = {}


class _Bacc(bacc.Bacc):
    """Bacc with activation-table thrash fix: restrict the table-set choices
    so Exp/Ln resolve to natural_log_exp_and_others and Gelu to
    gelu_and_others (set order/ids preserved; only contents filtered)."""

    _KEEP = {"natural_log_exp_and_others", "gelu_and_others"}

    def insert_act_table_loads(self):
        from concourse.hw_specs import get_activation_tables
        import bass_rust as _bass_rust

        has_activation = any(
            isinstance(i, mybir.InstActivation)
            for b in self.main_func.blocks
            for i in b.instructions
        )
        if not has_activation:
            return
        tables = [
            (name, fns if name in self._KEEP else set())
            for name, fns in get_activation_tables(self.m.arch).items()
        ]
        _bass_rust.insert_act_table_loads(self, tables)


DBG_SHAPES = {
    "xln": ([128, DC * TPC], dt.bfloat16),
    "q": ([128, DC * TPC], dt.bfloat16),
    "ksb": ([128, DC * SEQ], dt.bfloat16),
    "vsb": ([128, KC * VW], dt.bfloat16),
    "ot": ([128, DC * TPC], dt.bfloat16),
    "xattn": ([128, DC * TPC], dt.float32),
    "h1": ([128, FC * TPC], dt.bfloat16),
    "xl0": ([128, DC * TPC], dt.float32),
}


def _build(ln_affine: bool, masked: bool = False, dbg: str | None = None):
    nc = _Bacc("TRN2", target_bir_lowering=False, debug=False,
               num_devices=N_CORES)

    xT = nc.dram_tensor("xT", [DIM, TPC], dt.float32, kind="ExternalInput").ap()
    wqkv = nc.dram_tensor("wqkv", [DEPTH, DIM, 3 * INNER], dt.bfloat16,
                          kind="ExternalInput").ap()
    wo = nc.dram_tensor("wo", [DEPTH, INNER, DIM], dt.bfloat16,
                        kind="ExternalInput").ap()
    w1 = nc.dram_tensor("w1", [DEPTH, DIM, FF], dt.bfloat16,
                        kind="ExternalInput").ap()
    w2 = nc.dram_tensor("w2", [DEPTH, FF, DIM], dt.bfloat16,
                        kind="ExternalInput").ap()
    b1 = nc.dram_tensor("b1", [DEPTH, FF], dt.float32,
                        kind="ExternalInput").ap()
    b2 = nc.dram_tensor("b2", [DEPTH, DIM], dt.float32,
                        kind="ExternalInput").ap()
    if masked:
        # multiplicative key mask (1 visible / 0 masked), applied to V rows;
        # host pre-permutes to the gathered key-chunk order.
        amult = nc.dram_tensor("amult", [SEQ], dt.float32,
                               kind="ExternalInput").ap()
    if ln_affine:
        ln_g = nc.dram_tensor("ln_g", [2 * DEPTH, DIM], dt.float32,
                              kind="ExternalInput").ap()
        ln_b = nc.dram_tensor("ln_b", [2 * DEPTH, DIM], dt.float32,
                              kind="ExternalInput").ap()
    xO = nc.dram_tensor("xO", [DIM, TPC], dt.float32,
                        kind="ExternalOutput").ap()
    dbg_t = None
    if dbg is not None:
        shp, ddt = DBG_SHAPES[dbg]
        dbg_t = nc.dram_tensor("dbg", shp, ddt, kind="ExternalOutput").ap()

    groups = [[0, 1, 2, 3], [4, 5, 6, 7]]

    with tile.TileContext(nc) as tc:
        with (
            tc.tile_pool(name="pers", bufs=1) as pers,
            tc.tile_pool(name="wp", bufs=4) as wp,
            tc.tile_pool(name="tp2", bufs=2) as tp2,
            tc.tile_pool(name="tp3", bufs=3) as tp3,
            tc.tile_pool(name="es", bufs=4) as esp,
            tc.tile_pool(name="st", bufs=2) as stp,
            tc.tile_pool(name="pp", bufs=2, space="PSUM") as pp,
            tc.tile_pool(name="ppo", bufs=2, space="PSUM") as ppo,
            tc.tile_pool(name="pst", bufs=2, space="PSUM") as pst,
            tc.tile_pool(name="dram", bufs=2, space="DRAM") as dram,
        ):
            # ---- persistent tiles
            x_sb = pers.tile([128, DC * TPC], dt.float32)     # residual, d-major
            xln = pers.tile([128, DC * TPC], dt.bfloat16)     # ln output, d-major
            qbf = pers.tile([128, DC * TPC], dt.bfloat16)     # q, d-major
            ksb = pers.tile([128, DC * SEQ], dt.bfloat16)     # gathered k, d-major
            vsb = pers.tile([128, KC * VW], dt.bfloat16)      # gathered v + ones
            otsb = pers.tile([128, DC * TPC], dt.bfloat16)    # attn out^T, d-major
            h1sb = pers.tile([128, FC * TPC], dt.bfloat16)    # ffn hidden
            onesb = pers.tile([128, 1], dt.bfloat16)
            epsb = pers.tile([1, 1], dt.float32)
            b1sb = pers.tile([128, FC], dt.float32)
            b2sb = pers.tile([128, DC], dt.float32)
            absb = pers.tile([128, KC], dt.float32) if masked else None
            if ln_affine:
                lngsb = pers.tile([128, 2 * DEPTH * DC], dt.float32)
                lnbsb = pers.tile([128, 2 * DEPTH * DC], dt.float32)

            x_v = x_sb.rearrange("p (c t) -> p c t", c=DC)
            xln_v = xln.rearrange("p (c t) -> p c t", c=DC)
            q_v = qbf.rearrange("p (c t) -> p c t", c=DC)
            k_v = ksb.rearrange("p (c t) -> p c t", c=DC)
            vs_v = vsb.rearrange("p (g h e) -> p g h e", g=KC, h=HEADS, e=DH + 1)
            ot_v = otsb.rearrange("p (c t) -> p c t", c=DC)
            h1_v = h1sb.rearrange("p (f t) -> p f t", f=FC)

            nc.vector.memset(onesb[:], 1.0)
            nc.vector.memset(epsb[:], EPS)
            nc.gpsimd.memset(vs_v[:, :, :, DH:DH + 1], 1.0)
            nc.sync.dma_start(x_sb[:], xT.rearrange("(c p) t -> p c t", p=128))
            if masked:
                nc.sync.dma_start(
                    absb[:], amult.rearrange("(k p) -> p k", p=128))
            if ln_affine:
                nc.sync.dma_start(
                    lngsb[:], ln_g.rearrange("l (c p) -> p (l c)", p=128))
                nc.sync.dma_start(
                    lnbsb[:], ln_b.rearrange("l (c p) -> p (l c)", p=128))

            def layer_norm(l2, src_v, dst_v, t0=0, t1=TPC):
                """dst (bf16) = layernorm(src) along partition-major feature
                dim, for token range [t0, t1)."""
                W = t1 - t0
                psum_s = pst.tile([1, TPC], dt.float32, tag="lnps")
                psum_q = pst.tile([1, TPC], dt.float32, tag="lnps")
                for c in range(DC):
                    xb = tp3.tile([128, TPC], dt.bfloat16, tag="lncast")
                    nc.vector.tensor_copy(xb[:, 0:W], src_v[:, c, t0:t1])
                    x2 = tp3.tile([128, TPC], dt.bfloat16, tag="lnsq")
                    nc.vector.tensor_mul(x2[:, 0:W], xb[:, 0:W], xb[:, 0:W])
                    nc.tensor.matmul(psum_s[0:1, 0:W], onesb[:], xb[:, 0:W],
                                     start=(c == 0), stop=(c == DC - 1))
                    nc.tensor.matmul(psum_q[0:1, 0:W], onesb[:], x2[:, 0:W],
                                     start=(c == 0), stop=(c == DC - 1))
                mu = stp.tile([1, TPC], dt.float32, tag="stat")
                nc.vector.tensor_scalar_mul(mu[0:1, 0:W], psum_s[0:1, 0:W],
                                            1.0 / DIM)
                mub = tp2.tile([128, TPC], dt.float32, tag="mub")
                nc.gpsimd.partition_broadcast(mub[:, 0:W], mu[0:1, 0:W])
                negmusq = stp.tile([1, TPC], dt.float32, tag="stat")
                nc.vector.scalar_tensor_tensor(
                    out=negmusq[0:1, 0:W], in0=mu[0:1, 0:W], scalar=-1.0,
                    in1=mu[0:1, 0:W], op0=OP.mult, op1=OP.mult)
                var = stp.tile([1, TPC], dt.float32, tag="stat")
                nc.vector.scalar_tensor_tensor(
                    out=var[0:1, 0:W], in0=psum_q[0:1, 0:W], scalar=1.0 / DIM,
                    in1=negmusq[0:1, 0:W], op0=OP.mult, op1=OP.add)
                lnv = stp.tile([1, TPC], dt.float32, tag="stat")
                nc.scalar.activation(lnv[0:1, 0:W], var[0:1, 0:W], AF.Ln,
                                     bias=epsb[:])
                rstd = stp.tile([1, TPC], dt.float32, tag="stat")
                nc.scalar.activation(rstd[0:1, 0:W], lnv[0:1, 0:W], AF.Exp,
                                     scale=-0.5)
                rsb = tp2.tile([128, TPC], dt.float32, tag="rsb")
                nc.gpsimd.partition_broadcast(rsb[:, 0:W], rstd[0:1, 0:W])
                for c in range(DC):
                    t1t = tp3.tile([128, TPC], dt.float32, tag="lnt1")
                    nc.vector.tensor_sub(t1t[:, 0:W], src_v[:, c, t0:t1],
                                         mub[:, 0:W])
                    if ln_affine:
                        t2 = tp3.tile([128, TPC], dt.float32, tag="lnt2")
                        nc.vector.tensor_mul(t2[:, 0:W], t1t[:, 0:W],
                                             rsb[:, 0:W])
                        nc.vector.tensor_scalar(
                            dst_v[:, c, t0:t1], t2[:, 0:W],
                            lngsb[:, l2 * DC + c:l2 * DC + c + 1],
                            lnbsb[:, l2 * DC + c:l2 * DC + c + 1],
                            OP.mult, OP.add)
                    else:
                        nc.vector.tensor_mul(dst_v[:, c, t0:t1], t1t[:, 0:W],
                                             rsb[:, 0:W])

            def stream_mm_dmajor(w_src, col0, ncols, rhs_v, consume):
                """out[cols, tok] = W[:, col0:col0+ncols].T @ act, d-major act as
                rhs. consume(ps, jj) takes psum [128, TPC] for output-col chunk
                jj (128 cols each, numbered from col0/128)."""
                for s in range(ncols // WSL):
                    wt = wp.tile([128, DC * WSL], dt.bfloat16, tag="w")
                    nc.sync.dma_start(
                        wt[:], w_src[:, :, col0 + WSL * s:col0 + WSL * (s + 1)])
                    wt_v = wt.rearrange("p (c o) -> p c o", c=DC)
                    for j in range(WSL // 128):
                        ps = pp.tile([128, TPC], dt.float32, tag="mm")
                        for c in range(DC):
                            nc.tensor.matmul(
                                ps[:], wt_v[:, c, 128 * j:128 * (j + 1)],
                                rhs_v[:, c, :],
                                start=(c == 0), stop=(c == DC - 1))
                        consume(ps, (col0 + WSL * s) // 128 + j)

            def dump(name, src_ap, l):
                if dbg == name and l == 0:
                    nc.sync.dma_start(dbg_t[:], src_ap)

            kvag = {}

            def ln1_kv_half(l, h):
                """ln1 for token half h of layer l, then K/V for that half and
                its AllGather trigger. Emitted inside layer l-1's FFN2 so the
                collective wire rides under compute."""
                if l not in kvag:
                    kvag[l] = (
                        [dram.tile([2 * KV_H], dt.bfloat16,
                                   name=f"kvin{l}h{hh}", tag=f"kvin{hh}")
                         for hh in range(2)],
                        [dram.tile([GROUP * 2 * KV_H], dt.bfloat16,
                                   name=f"kvag{l}h{hh}", tag=f"kvag{hh}")
                         for hh in range(2)])
                kv_in, ag = kvag[l]
                with nc.named_scope(f"L{l}_ln1h{h}"):
                    layer_norm(2 * l, x_v, xln_v, HT * h, HT * (h + 1))
                wq = wqkv[l].rearrange("(c p) o -> p c o", p=128)
                with nc.named_scope(f"L{l}_kv{h}"):
                    k_dst = kv_in[h][0:KV_H].rearrange("(r t) -> r t", t=HT)
                    for s in range(INNER // WSL):
                        wt = wp.tile([128, DC * WSL], dt.bfloat16, tag="w")
                        nc.sync.dma_start(
                            wt[:],
                            wq[:, :, INNER + WSL * s:INNER + WSL * (s + 1)])
                        wt_v = wt.rearrange("p (c o) -> p c o", c=DC)
                        for j in range(WSL // 128):
                            ps = pp.tile([128, TPC], dt.float32, tag="mm")
                            for c in range(DC):
                                nc.tensor.matmul(
                                    ps[:, 0:HT],
                                    wt_v[:, c, 128 * j:128 * (j + 1)],
                                    xln_v[:, c, HT * h:HT * (h + 1)],
                                    start=(c == 0), stop=(c == DC - 1))
                            kown = tp3.tile([128, HT], dt.bfloat16,
                                            tag="kv_own")
                            nc.vector.tensor_copy(kown[:], ps[:, 0:HT])
                            r = 128 * (2 * s + j)
                            nc.scalar.dma_start(k_dst[r:r + 128, :], kown[:])
                    # V for this half: token-major out
                    v_dst = kv_in[h][KV_H:2 * KV_H].rearrange(
                        "(t v) -> t v", v=INNER)
                    for s in range(INNER // WSL):
                        wt = wp.tile([128, DC * WSL], dt.bfloat16, tag="w")
                        nc.sync.dma_start(
                            wt[:],
                            wq[:, :, 2 * INNER + WSL * s:
                               2 * INNER + WSL * (s + 1)])
                        wt_v = wt.rearrange("p (c o) -> p c o", c=DC)
                        for t in range(2):
                            ps = pp.tile([128, TPC], dt.float32, tag="mm")
                            for c in range(DC):
                                nc.tensor.matmul(
                                    ps[:, 0:WSL],
                                    xln_v[:, c,
                                          HT * h + 128 * t:
                                          HT * h + 128 * (t + 1)],
                                    wt_v[:, c, :],
                                    start=(c == 0), stop=(c == DC - 1))
                            vown = tp3.tile([128, WSL], dt.bfloat16,
                                            tag="v_own")
                            nc.vector.tensor_copy(vown[:], ps[:, 0:WSL])
                            nc.scalar.dma_start(
                                v_dst[128 * t:128 * (t + 1),
                                      WSL * s:WSL * (s + 1)], vown[:])
                    nc.gpsimd.collective_compute(
                        "AllGather", OP.bypass,
                        ins=[kv_in[h].opt()], outs=[ag[h].opt()],
                        replica_groups=groups)

            for l in range(DEPTH):
                for h in range(2):
                    ln1_kv_half(l, h)
                kv_in, ag = kvag[l]
                wq = wqkv[l].rearrange("(c p) o -> p c o", p=128)
                dump("xln", xln[:], l)

                # ---- q for own tokens (overlaps the AllGathers)
                with nc.named_scope(f"L{l}_q"):
                    def q_consume(ps, jj):
                        nc.vector.tensor_copy(q_v[:, jj, :], ps[:])
                    stream_mm_dmajor(wq, 0, INNER, xln_v, q_consume)
                dump("q", qbf[:], l)

                # ---- load gathered k, v; key chunk ci = h*8 + r*2 + g
                with nc.named_scope(f"L{l}_kvload"):
                    for h in range(2):
                        for r in range(GROUP):
                            base = r * 2 * KV_H
                            off = (h * GROUP + r) * HT
                            k_src = ag[h][base:base + KV_H].rearrange(
                                "(c p t) -> p c t", p=128, t=HT)
                            nc.gpsimd.dma_start(
                                k_v[:, :, off:off + HT], k_src)
                            v_src = ag[h][base + KV_H:base + 2 * KV_H].rearrange(
                                "(g p hh e) -> p g hh e", p=128, hh=HEADS, e=DH)
                            for g in range(HT // 128):
                                nc.scalar.dma_start(
                                    vs_v[:, off // 128 + g, :, 0:DH],
                                    v_src[:, g])
                    if masked:
                        vv = vsb.rearrange("p (g w) -> p g w", g=KC)
                        for kc in range(KC):
                            nc.vector.tensor_scalar_mul(
                                vv[:, kc, :], vv[:, kc, :],
                                absb[:, kc:kc + 1])
                dump("ksb", ksb[:], l)
                dump("vsb", vsb[:], l)

                # ---- attention, head-pair at a time; the two 64-row score
                # matmuls of a pair run concurrently in disjoint PE row groups.
                with nc.named_scope(f"L{l}_attn"):
                    for hc in range(HEADS // 2):
                        h0, h1 = 2 * hc, 2 * hc + 1
                        po_a = ppo.tile([128, TPC], dt.float32, tag="attno")
                        po_b = ppo.tile([128, TPC], dt.float32, tag="attno")
                        es_q = [None] * KC

                        def attn_v(kc):
                            es2 = es_q[kc]
                            nc.tensor.matmul(
                                po_a[0:DH + 1, :],
                                vsb[:, kc * VW + h0 * (DH + 1):
                                    kc * VW + (h0 + 1) * (DH + 1)],
                                es2[:, 0:TPC],
                                start=(kc == 0), stop=(kc == KC - 1))
                            nc.tensor.matmul(
                                po_b[0:DH + 1, :],
                                vsb[:, kc * VW + h1 * (DH + 1):
                                    kc * VW + (h1 + 1) * (DH + 1)],
                                es2[:, TPC:2 * TPC],
                                start=(kc == 0), stop=(kc == KC - 1))

                        # skew-2 software pipeline: attnV(kc-2) issues after
                        # scores(kc) so the PE FIFO never waits on exp.
                        for kc in range(KC):
                            if kc >= 2:
                                attn_v(kc - 2)
                            ps2 = pp.tile([128, 2 * TPC], dt.float32, tag="mm")
                            nc.tensor.matmul(
                                ps2[:, 0:TPC],
                                k_v[0:DH, hc, 128 * kc:128 * (kc + 1)],
                                q_v[0:DH, hc, :],
                                start=True, stop=True, tile_position=(0, 0))
                            nc.tensor.matmul(
                                ps2[:, TPC:2 * TPC],
                                k_v[DH:128, hc, 128 * kc:128 * (kc + 1)],
                                q_v[DH:128, hc, :],
                                start=True, stop=True, tile_position=(64, 0))
                            es2 = esp.tile([128, 2 * TPC], dt.bfloat16,
                                           tag="es")
                            nc.scalar.activation(es2[:], ps2[:], AF.Exp,
                                                 scale=DH ** -0.5)
                            es_q[kc] = es2
                        attn_v(KC - 2)
                        attn_v(KC - 1)
                        # evacuate PSUM promptly (frees po for the next pair),
                        # then normalize from SBUF off the critical path.
                        for po, h in ((po_a, h0), (po_b, h1)):
                            hp = 64 * (h % 2)
                            pot = tp2.tile([DH + 1, TPC], dt.float32,
                                           tag="pot")
                            nc.vector.tensor_copy(pot[:], po[0:DH + 1, :])
                            lnd = stp.tile([1, TPC], dt.float32, tag="lnd")
                            nc.scalar.activation(lnd[:], pot[DH:DH + 1, :],
                                                 AF.Ln)
                            rec = stp.tile([1, TPC], dt.float32, tag="rec")
                            nc.scalar.activation(rec[:], lnd[:], AF.Exp,
                                                 scale=-1.0)
                            bc = tp2.tile([64, TPC], dt.float32, tag="bc")
                            nc.gpsimd.partition_broadcast(bc[:], rec[:])
                            nc.vector.tensor_mul(
                                ot_v[hp:hp + DH, hc, :], pot[0:DH, :], bc[:])
                dump("ot", otsb[:], l)

                # ---- wo + residual
                with nc.named_scope(f"L{l}_wo"):
                    wov = wo[l].rearrange("(c p) o -> p c o", p=128)

                    def wo_consume(ps, jj):
                        nc.vector.tensor_add(x_v[:, jj, :], x_v[:, jj, :], ps[:])
                    stream_mm_dmajor(wov, 0, DIM, ot_v, wo_consume)
                dump("xattn", x_sb[:], l)

                with nc.named_scope(f"L{l}_ln2"):
                    layer_norm(2 * l + 1, x_v, xln_v)

                # ---- FFN
                with nc.named_scope(f"L{l}_ffn1"):
                    nc.sync.dma_start(
                        b1sb[:], b1[l].rearrange("(f p) -> p f", p=128))
                    w1v = w1[l].rearrange("(c p) o -> p c o", p=128)

                    def ffn1_consume(ps, f):
                        nc.scalar.activation(
                            h1_v[:, f, :], ps[:], AF.Gelu,
                            bias=b1sb[:, f:f + 1])
                    stream_mm_dmajor(w1v, 0, FF, xln_v, ffn1_consume)
                dump("h1", h1sb[:], l)

                nc.sync.dma_start(
                    b2sb[:], b2[l].rearrange("(c p) -> p c", p=128))
                w2v = w2[l].rearrange("(f p) o -> p f o", p=128)

                with nc.named_scope(f"L{l}_ffn2"):
                    for j in range(DC):
                        ps = ppo.tile([128, TPC], dt.float32, tag="attno")
                        for half in range(2):
                            wt = wp.tile([128, (FC // 2) * 128], dt.bfloat16,
                                         tag="w")
                            nc.sync.dma_start(
                                wt[:],
                                w2v[:, (FC // 2) * half:(FC // 2) * (half + 1),
                                    128 * j:128 * (j + 1)])
                            wt_v = wt.rearrange("p (f o) -> p f o", f=FC // 2)
                            for f2 in range(FC // 2):
                                f = (FC // 2) * half + f2
                                nc.tensor.matmul(
                                    ps[:], wt_v[:, f2, :], h1_v[:, f, :],
                                    start=(f == 0), stop=(f == FC - 1))
                        # x += psum + b2  (b2 is per-feature = per-partition)
                        t3 = tp2.tile([128, TPC], dt.float32, tag="res2")
                        nc.vector.tensor_scalar_add(
                            t3[:], ps[:], b2sb[:, j:j + 1])
                        nc.vector.tensor_add(x_v[:, j, :], x_v[:, j, :], t3[:])
                dump("xl0", x_sb[:], l)

            nc.sync.dma_start(xO.rearrange("(c p) t -> p c t", p=128),
                              x_v[:, :, :])

    nc.compile()
    return nc


def _prepare(inputs):
    x = np.ascontiguousarray(np.asarray(inputs["x"], dtype=np.float32))
    mask = np.asarray(inputs["mask"])
    wqkv = np.asarray(inputs["wqkv"], dtype=np.float32)
    wo = np.asarray(inputs["wo"], dtype=np.float32)
    w1 = np.asarray(inputs["w1"], dtype=np.float32)
    w2 = np.asarray(inputs["w2"], dtype=np.float32)
    b1 = np.asarray(inputs["b1"], dtype=np.float32)
    b2 = np.asarray(inputs["b2"], dtype=np.float32)
    ln1_g = np.asarray(inputs["ln1_g"], dtype=np.float32)
    ln1_b = np.asarray(inputs["ln1_b"], dtype=np.float32)
    ln2_g = np.asarray(inputs["ln2_g"], dtype=np.float32)
    ln2_b = np.asarray(inputs["ln2_b"], dtype=np.float32)

    ln_affine = not (
        np.all(ln1_g == 1.0) and np.all(ln1_b == 0.0)
        and np.all(ln2_g == 1.0) and np.all(ln2_b == 0.0))
    mask_b = np.asarray(mask, bool)
    masked = not np.all(mask_b)

    key = (ln_affine, masked)
    if key not in _CACHE:
        _CACHE[key] = _build(ln_affine, masked)
    nc = _CACHE[key]

    wqkv_bf = np.ascontiguousarray(wqkv.astype(BF16))
    wo_bf = np.ascontiguousarray(wo.astype(BF16))
    w1_bf = np.ascontiguousarray(w1.astype(BF16))
    w2_bf = np.ascontiguousarray(w2.astype(BF16))
    amult = mask_b.astype(np.float32)   # 1 visible / 0 masked

    in_maps = []
    for c in range(N_CORES):
        b = c // GROUP
        s = c % GROUP
        xT = np.ascontiguousarray(x[b, s * TPC:(s + 1) * TPC, :].T)
        m = {
            "xT": xT, "wqkv": wqkv_bf, "wo": wo_bf, "w1": w1_bf, "w2": w2_bf,
            "b1": b1, "b2": b2,
        }
        if masked:
            # permute keys to gathered chunk order: ci = h*8 + r*2 + g
            am = np.empty(SEQ, np.float32)
            ci = 0
            for h in range(2):
                for r in range(GROUP):
                    for g in range(HT // 128):
                        src = r * TPC + h * HT + g * 128
                        am[ci * 128:(ci + 1) * 128] = amult[b, src:src + 128]
                        ci += 1
            m["amult"] = np.ascontiguousarray(am)
        if ln_affine:
            g = np.stack([v for pair in zip(ln1_g, ln2_g) for v in pair])
            bb = np.stack([v for pair in zip(ln1_b, ln2_b) for v in pair])
            m["ln_g"] = np.ascontiguousarray(g)
            m["ln_b"] = np.ascontiguousarray(bb)
        in_maps.append(m)
    return nc, in_maps


def _assemble(results):
    out = np.empty((BATCH, SEQ, DIM), np.float32)
    for c in range(N_CORES):
        b = c // GROUP
        s = c % GROUP
        out[b, s * TPC:(s + 1) * TPC, :] = results[c]["xO"].T
    return out


def kernel(**inputs):
    nc, in_maps = _prepare(inputs)
    res = run_bass_kernel_spmd(nc, in_maps, list(range(N_CORES)))
    return _assemble(res.results)


def run_traced(**inputs):
    """Used by test.py only: run with NTFF profiling enabled."""
    nc, in_maps = _prepare(inputs)
    return run_bass_kernel_spmd(nc, in_maps, list(range(N_CORES)), trace=True)


# revision 14
# speedup vs baseline: 1.1017x; 1.1017x over previous
"""Trainium2 Bass kernel for a 4-layer transformer encoder.

Sharding: 8-way data-parallel over tokens. Cores 0-3 handle batch element 0,
cores 4-7 batch element 1; each core owns a 512-token slice of its batch
element's 2048-token sequence. Attention needs full-sequence K/V, so each
layer AllGathers the core's K/V slice within its 4-core group — split into
two token-half collectives so the wire time overlaps KV/Q compute and the
gathered loads pipeline.

On-chip layout: activations are feature-major ([d, tokens]); LayerNorm
statistics are computed with ones-vector matmuls (partition-axis reduction),
softmax runs on transposed scores ([keys, queries]) so the denominator comes
from an appended ones-column in V and no transposes are needed anywhere.
Attention processes heads in pairs: the two 64-row score matmuls of a pair
run concurrently in disjoint PE row-groups (tile_position packing), which
keeps the full 128x128 array active. Matmuls run in bf16 with fp32 PSUM
accumulation; the residual stream stays fp32 in SBUF.
"""
import numpy as np
import ml_dtypes

import concourse.bass as bass
import concourse.bacc as bacc
import concourse.tile as tile
import concourse.mybir as mybir
from concourse.bass_utils import run_bass_kernel_spmd

dt = mybir.dt
AF = mybir.ActivationFunctionType
OP = mybir.AluOpType
BF16 = ml_dtypes.bfloat16

DIM = 1024
DEPTH = 4
HEADS = 16
DH = 64
INNER = 1024
FF = 4096
EPS = 1e-5
SEQ = 2048
BATCH = 2
N_CORES = 8
GROUP = 4                 # cores per batch element
TPC = SEQ // GROUP        # tokens per core = 512
HT = TPC // 2             # tokens per AG half = 256
DC = DIM // 128           # 8 dmodel chunks
KC = SEQ // 128           # 16 key chunks
FC = FF // 128            # 32 ff chunks
VW = HEADS * (DH + 1)     # v tile row width per key chunk (ones col appended)
KV_H = DIM * HT           # elements in one k (or v) half-slice
WSL = 256                 # weight slice column width

_CACHE = {}


class _Bacc(bacc.Bacc):
    """Bacc with activation-table thrash fix: restrict the table-set choices
    so Exp/Ln resolve to natural_log_exp_and_others and Gelu to
    gelu_and_others (set order/ids preserved; only contents filtered)."""

    _KEEP = {"natural_log_exp_and_others", "gelu_and_others"}

    def insert_act_table_loads(self):
        from concourse.hw_specs import get_activation_tables
        import bass_rust as _bass_rust

        has_activation = any(
            isinstance(i, mybir.InstActivation)
            for b in self.main_func.blocks
            for i in b.instructions
        )
        if not has_activation:
            return
        tables = [
            (name, fns if name in self._KEEP else set())
            for name, fns in get_activation_tables(self.m.arch).items()
        ]
        _bass_rust.insert_act_table_loads(self, tables)


DBG_SHAPES = {
    "xln": ([128, DC * TPC], dt.bfloat16),
    "q": ([128, DC * TPC], dt.bfloat16),
    "ksb": ([128, DC * SEQ], dt.bfloat16),
    "vsb": ([128, KC * VW], dt.bfloat16),
    "ot": ([128, DC * TPC], dt.bfloat16),
    "xattn": ([128, DC * TPC], dt.float32),
    "h1": ([128, FC * TPC], dt.bfloat16),
    "xl0": ([128, DC * TPC], dt.float32),
}


def _build(ln_affine: bool, masked: bool = False, dbg: str | None = None):
    nc = _Bacc("TRN2", target_bir_lowering=False, debug=False,
               num_devices=N_CORES)

    xT = nc.dram_tensor("xT", [DIM, TPC], dt.float32, kind="ExternalInput").ap()
    wqkv = nc.dram_tensor("wqkv", [DEPTH, DIM, 3 * INNER], dt.bfloat16,
                          kind="ExternalInput").ap()
    wo = nc.dram_tensor("wo", [DEPTH, INNER, DIM], dt.bfloat16,
                        kind="ExternalInput").ap()
    w1 = nc.dram_tensor("w1", [DEPTH, DIM, FF], dt.bfloat16,
                        kind="ExternalInput").ap()
    w2 = nc.dram_tensor("w2", [DEPTH, FF, DIM], dt.bfloat16,
                        kind="ExternalInput").ap()
    b1 = nc.dram_tensor("b1", [DEPTH, FF], dt.float32,
                        kind="ExternalInput").ap()
    b2 = nc.dram_tensor("b2", [DEPTH, DIM], dt.float32,
                        kind="ExternalInput").ap()
    if masked:
        # multiplicative key mask (1 visible / 0 masked), applied to V rows;
        # host pre-permutes to the gathered key-chunk order.
        amult = nc.dram_tensor("amult", [SEQ], dt.float32,
                               kind="ExternalInput").ap()
    if ln_affine:
        ln_g = nc.dram_tensor("ln_g", [2 * DEPTH, DIM], dt.float32,
                              kind="ExternalInput").ap()
        ln_b = nc.dram_tensor("ln_b", [2 * DEPTH, DIM], dt.float32,
                              kind="ExternalInput").ap()
    xO = nc.dram_tensor("xO", [DIM, TPC], dt.float32,
                        kind="ExternalOutput").ap()
    dbg_t = None
    if dbg is not None:
        shp, ddt = DBG_SHAPES[dbg]
        dbg_t = nc.dram_tensor("dbg", shp, ddt, kind="ExternalOutput").ap()

    groups = [[0, 1, 2, 3], [4, 5, 6, 7]]

    with tile.TileContext(nc) as tc:
        with (
            tc.tile_pool(name="pers", bufs=1) as pers,
            tc.tile_pool(name="wp", bufs=4) as wp,
            tc.tile_pool(name="tp2", bufs=2) as tp2,
            tc.tile_pool(name="tp3", bufs=3) as tp3,
            tc.tile_pool(name="es", bufs=4) as esp,
            tc.tile_pool(name="st", bufs=2) as stp,
            tc.tile_pool(name="pp", bufs=2, space="PSUM") as pp,
            tc.tile_pool(name="ppo", bufs=2, space="PSUM") as ppo,
            tc.tile_pool(name="pst", bufs=2, space="PSUM") as pst,
            tc.tile_pool(name="dram", bufs=2, space="DRAM") as dram,
        ):
            # ---- persistent tiles
            x_sb = pers.tile([128, DC * TPC], dt.float32)     # residual, d-major
            xln = pers.tile([128, DC * TPC], dt.bfloat16)     # ln output, d-major
            qbf = pers.tile([128, DC * TPC], dt.bfloat16)     # q, d-major
            ksb = pers.tile([128, DC * SEQ], dt.bfloat16)     # gathered k, d-major
            vsb = pers.tile([128, KC * VW], dt.bfloat16)      # gathered v + ones
            otsb = pers.tile([128, DC * TPC], dt.bfloat16)    # attn out^T, d-major
            h1sb = pers.tile([128, FC * TPC], dt.bfloat16)    # ffn hidden
            onesb = pers.tile([128, 1], dt.bfloat16)
            epsb = pers.tile([1, 1], dt.float32)
            b1sb = pers.tile([128, FC], dt.float32)
            b2sb = pers.tile([128, DC], dt.float32)
            absb = pers.tile([128, KC], dt.float32) if masked else None
            if ln_affine:
                lngsb = pers.tile([128, 2 * DEPTH * DC], dt.float32)
                lnbsb = pers.tile([128, 2 * DEPTH * DC], dt.float32)

            x_v = x_sb.rearrange("p (c t) -> p c t", c=DC)
            xln_v = xln.rearrange("p (c t) -> p c t", c=DC)
            q_v = qbf.rearrange("p (c t) -> p c t", c=DC)
            k_v = ksb.rearrange("p (c t) -> p c t", c=DC)
            vs_v = vsb.rearrange("p (g h e) -> p g h e", g=KC, h=HEADS, e=DH + 1)
            ot_v = otsb.rearrange("p (c t) -> p c t", c=DC)
            h1_v = h1sb.rearrange("p (f t) -> p f t", f=FC)

            nc.vector.memset(onesb[:], 1.0)
            nc.vector.memset(epsb[:], EPS)
            nc.gpsimd.memset(vs_v[:, :, :, DH:DH + 1], 1.0)
            nc.sync.dma_start(x_sb[:], xT.rearrange("(c p) t -> p c t", p=128))
            if masked:
                nc.sync.dma_start(
                    absb[:], amult.rearrange("(k p) -> p k", p=128))
            if ln_affine:
                nc.sync.dma_start(
                    lngsb[:], ln_g.rearrange("l (c p) -> p (l c)", p=128))
                nc.sync.dma_start(
                    lnbsb[:], ln_b.rearrange("l (c p) -> p (l c)", p=128))

            def layer_norm(l2, src_v, dst_v, t0=0, t1=TPC):
                """dst (bf16) = layernorm(src) along partition-major feature
                dim, for token range [t0, t1)."""
                W = t1 - t0
                psum_s = pst.tile([1, TPC], dt.float32, tag="lnps")
                psum_q = pst.tile([1, TPC], dt.float32, tag="lnps")
                for c in range(DC):
                    xb = tp3.tile([128, TPC], dt.bfloat16, tag="lncast")
                    nc.vector.tensor_copy(xb[:, 0:W], src_v[:, c, t0:t1])
                    x2 = tp3.tile([128, TPC], dt.bfloat16, tag="lnsq")
                    nc.vector.tensor_mul(x2[:, 0:W], xb[:, 0:W], xb[:, 0:W])
                    nc.tensor.matmul(psum_s[0:1, 0:W], onesb[:], xb[:, 0:W],
                                     start=(c == 0), stop=(c == DC - 1))
                    nc.tensor.matmul(psum_q[0:1, 0:W], onesb[:], x2[:, 0:W],
                                     start=(c == 0), stop=(c == DC - 1))
                mu = stp.tile([1, TPC], dt.float32, tag="stat")
                nc.vector.tensor_scalar_mul(mu[0:1, 0:W], psum_s[0:1, 0:W],
                                            1.0 / DIM)
                mub = tp2.tile([128, TPC], dt.float32, tag="mub")
                nc.gpsimd.partition_broadcast(mub[:, 0:W], mu[0:1, 0:W])
                negmusq = stp.tile([1, TPC], dt.float32, tag="stat")
                nc.vector.scalar_tensor_tensor(
                    out=negmusq[0:1, 0:W], in0=mu[0:1, 0:W], scalar=-1.0,
                    in1=mu[0:1, 0:W], op0=OP.mult, op1=OP.mult)
                var = stp.tile([1, TPC], dt.float32, tag="stat")
                nc.vector.scalar_tensor_tensor(
                    out=var[0:1, 0:W], in0=psum_q[0:1, 0:W], scalar=1.0 / DIM,
                    in1=negmusq[0:1, 0:W], op0=OP.mult, op1=OP.add)
                lnv = stp.tile([1, TPC], dt.float32, tag="stat")
                nc.scalar.activation(lnv[0:1, 0:W], var[0:1, 0:W], AF.Ln,
                                     bias=epsb[:])
                rstd = stp.tile([1, TPC], dt.float32, tag="stat")
                nc.scalar.activation(rstd[0:1, 0:W], lnv[0:1, 0:W], AF.Exp,
                                     scale=-0.5)
                rsb = tp2.tile([128, TPC], dt.float32, tag="rsb")
                nc.gpsimd.partition_broadcast(rsb[:, 0:W], rstd[0:1, 0:W])
                for c in range(DC):
                    t1t = tp3.tile([128, TPC], dt.float32, tag="lnt1")
                    nc.vector.tensor_sub(t1t[:, 0:W], src_v[:, c, t0:t1],
                                         mub[:, 0:W])
                    if ln_affine:
                        t2 = tp3.tile([128, TPC], dt.float32, tag="lnt2")
                        nc.vector.tensor_mul(t2[:, 0:W], t1t[:, 0:W],
                                             rsb[:, 0:W])
                        nc.vector.tensor_scalar(
                            dst_v[:, c, t0:t1], t2[:, 0:W],
                            lngsb[:, l2 * DC + c:l2 * DC + c + 1],
                            lnbsb[:, l2 * DC + c:l2 * DC + c + 1],
                            OP.mult, OP.add)
                    else:
                        nc.vector.tensor_mul(dst_v[:, c, t0:t1], t1t[:, 0:W],
                                             rsb[:, 0:W])

            def stream_mm_dmajor(w_src, col0, ncols, rhs_v, consume):
                """out[cols, tok] = W[:, col0:col0+ncols].T @ act, d-major act as
                rhs. consume(ps, jj) takes psum [128, TPC] for output-col chunk
                jj (128 cols each, numbered from col0/128)."""
                for s in range(ncols // WSL):
                    wt = wp.tile([128, DC * WSL], dt.bfloat16, tag="w")
                    nc.sync.dma_start(
                        wt[:], w_src[:, :, col0 + WSL * s:col0 + WSL * (s + 1)])
                    wt_v = wt.rearrange("p (c o) -> p c o", c=DC)
                    for j in range(WSL // 128):
                        ps = pp.tile([128, TPC], dt.float32, tag="mm")
                        for c in range(DC):
                            nc.tensor.matmul(
                                ps[:], wt_v[:, c, 128 * j:128 * (j + 1)],
                                rhs_v[:, c, :],
                                start=(c == 0), stop=(c == DC - 1))
                        consume(ps, (col0 + WSL * s) // 128 + j)

            def dump(name, src_ap, l):
                if dbg == name and l == 0:
                    nc.sync.dma_start(dbg_t[:], src_ap)

            kvag = {}

            def ln1_kv_half(l, h):
                """ln1 for token half h of layer l, then K/V for that half and
                its AllGather trigger. Emitted inside layer l-1's FFN2 so the
                collective wire rides under compute."""
                if l not in kvag:
                    kvag[l] = (
                        [dram.tile([2 * KV_H], dt.bfloat16,
                                   name=f"kvin{l}h{hh}", tag=f"kvin{hh}")
                         for hh in range(2)],
                        [dram.tile([GROUP * 2 * KV_H], dt.bfloat16,
                                   name=f"kvag{l}h{hh}", tag=f"kvag{hh}")
                         for hh in range(2)])
                kv_in, ag = kvag[l]
                with nc.named_scope(f"L{l}_ln1h{h}"):
                    layer_norm(2 * l, x_v, xln_v, HT * h, HT * (h + 1))
                wq = wqkv[l].rearrange("(c p) o -> p c o", p=128)
                with nc.named_scope(f"L{l}_kv{h}"):
                    k_dst = kv_in[h][0:KV_H].rearrange("(r t) -> r t", t=HT)
                    for s in range(INNER // WSL):
                        wt = wp.tile([128, DC * WSL], dt.bfloat16, tag="w")
                        nc.sync.dma_start(
                            wt[:],
                            wq[:, :, INNER + WSL * s:INNER + WSL * (s + 1)])
                        wt_v = wt.rearrange("p (c o) -> p c o", c=DC)
                        for j in range(WSL // 128):
                            ps = pp.tile([128, TPC], dt.float32, tag="mm")
                            for c in range(DC):
                                nc.tensor.matmul(
                                    ps[:, 0:HT],
                                    wt_v[:, c, 128 * j:128 * (j + 1)],
                                    xln_v[:, c, HT * h:HT * (h + 1)],
                                    start=(c == 0), stop=(c == DC - 1))
                            kown = tp3.tile([128, HT], dt.bfloat16,
                                            tag="kv_own")
                            nc.vector.tensor_copy(kown[:], ps[:, 0:HT])
                            r = 128 * (2 * s + j)
                            nc.scalar.dma_start(k_dst[r:r + 128, :], kown[:])
                    # V for this half: token-major out
                    v_dst = kv_in[h][KV_H:2 * KV_H].rearrange(
                        "(t v) -> t v", v=INNER)
                    for s in range(INNER // WSL):
                        wt = wp.tile([128, DC * WSL], dt.bfloat16, tag="w")
                        nc.sync.dma_start(
                            wt[:],
                            wq[:, :, 2 * INNER + WSL * s:
                               2 * INNER + WSL * (s + 1)])
                        wt_v = wt.rearrange("p (c o) -> p c o", c=DC)
                        for t in range(2):
                            ps = pp.tile([128, TPC], dt.float32, tag="mm")
                            for c in range(DC):
                                nc.tensor.matmul(
                                    ps[:, 0:WSL],
                                    xln_v[:, c,
                                          HT * h + 128 * t:
                                          HT * h + 128 * (t + 1)],
                                    wt_v[:, c, :],
                                    start=(c == 0), stop=(c == DC - 1))
                            vown = tp3.tile([128, WSL], dt.bfloat16,
                                            tag="v_own")
                            nc.vector.tensor_copy(vown[:], ps[:, 0:WSL])
                            nc.scalar.dma_start(
                                v_dst[128 * t:128 * (t + 1),
                                      WSL * s:WSL * (s + 1)], vown[:])
                    nc.gpsimd.collective_compute(
                        "AllGather", OP.bypass,
                        ins=[kv_in[h].opt()], outs=[ag[h].opt()],
                        replica_groups=groups)

            for l in range(DEPTH):
                for h in range(2):
                    ln1_kv_half(l, h)
                kv_in, ag = kvag[l]
                wq = wqkv[l].rearrange("(c p) o -> p c o", p=128)
                dump("xln", xln[:], l)

                # ---- q for own tokens (overlaps the AllGathers)
                with nc.named_scope(f"L{l}_q"):
                    def q_consume(ps, jj):
                        nc.vector.tensor_copy(q_v[:, jj, :], ps[:])
                    stream_mm_dmajor(wq, 0, INNER, xln_v, q_consume)
                dump("q", qbf[:], l)

                # ---- load gathered k, v; key chunk ci = h*8 + r*2 + g
                with nc.named_scope(f"L{l}_kvload"):
                    for h in range(2):
                        for r in range(GROUP):
                            base = r * 2 * KV_H
                            off = (h * GROUP + r) * HT
                            k_src = ag[h][base:base + KV_H].rearrange(
                                "(c p t) -> p c t", p=128, t=HT)
                            nc.gpsimd.dma_start(
                                k_v[:, :, off:off + HT], k_src)
                            v_src = ag[h][base + KV_H:base + 2 * KV_H].rearrange(
                                "(g p hh e) -> p g hh e", p=128, hh=HEADS, e=DH)
                            for g in range(HT // 128):
                                nc.gpsimd.dma_start(
                                    vs_v[:, off // 128 + g, :, 0:DH],
                                    v_src[:, g])
                    if masked:
                        vv = vsb.rearrange("p (g w) -> p g w", g=KC)
                        for kc in range(KC):
                            nc.vector.tensor_scalar_mul(
                                vv[:, kc, :], vv[:, kc, :],
                                absb[:, kc:kc + 1])
                dump("ksb", ksb[:], l)
                dump("vsb", vsb[:], l)

                # ---- attention, head-pair at a time; the two 64-row score
                # matmuls of a pair run concurrently in disjoint PE row groups.
                with nc.named_scope(f"L{l}_attn"):
                    for hc in range(HEADS // 2):
                        h0, h1 = 2 * hc, 2 * hc + 1
                        po_a = ppo.tile([128, TPC], dt.float32, tag="attno")
                        po_b = ppo.tile([128, TPC], dt.float32, tag="attno")
                        es_q = [None] * KC

                        def attn_v(kc):
                            es2 = es_q[kc]
                            nc.tensor.matmul(
                                po_a[0:DH + 1, :],
                                vsb[:, kc * VW + h0 * (DH + 1):
                                    kc * VW + (h0 + 1) * (DH + 1)],
                                es2[:, 0:TPC],
                                start=(kc == 0), stop=(kc == KC - 1))
                            nc.tensor.matmul(
                                po_b[0:DH + 1, :],
                                vsb[:, kc * VW + h1 * (DH + 1):
                                    kc * VW + (h1 + 1) * (DH + 1)],
                                es2[:, TPC:2 * TPC],
                                start=(kc == 0), stop=(kc == KC - 1))

                        # skew-2 software pipeline: attnV(kc-2) issues after
                        # scores(kc) so the PE FIFO never waits on exp.
                        for kc in range(KC):
                            if kc >= 2:
                                attn_v(kc - 2)
                            ps2 = pp.tile([128, 2 * TPC], dt.float32, tag="mm")
                            nc.tensor.matmul(
                                ps2[:, 0:TPC],
                                k_v[0:DH, hc, 128 * kc:128 * (kc + 1)],
                                q_v[0:DH, hc, :],
                                start=True, stop=True, tile_position=(0, 0))
                            nc.tensor.matmul(
                                ps2[:, TPC:2 * TPC],
                                k_v[DH:128, hc, 128 * kc:128 * (kc + 1)],
                                q_v[DH:128, hc, :],
                                start=True, stop=True, tile_position=(64, 0))
                            es2 = esp.tile([128, 2 * TPC], dt.bfloat16,
                                           tag="es")
                            nc.scalar.activation(es2[:], ps2[:], AF.Exp,
                                                 scale=DH ** -0.5)
                            es_q[kc] = es2
                        attn_v(KC - 2)
                        attn_v(KC - 1)
                        # evacuate PSUM promptly (frees po for the next pair),
                        # then normalize from SBUF off the critical path.
                        for po, h in ((po_a, h0), (po_b, h1)):
                            hp = 64 * (h % 2)
                            pot = tp2.tile([DH + 1, TPC], dt.float32,
                                           tag="pot")
                            nc.vector.tensor_copy(pot[:], po[0:DH + 1, :])
                            lnd = stp.tile([1, TPC], dt.float32, tag="lnd")
                            nc.scalar.activation(lnd[:], pot[DH:DH + 1, :],
                                                 AF.Ln)
                            rec = stp.tile([1, TPC], dt.float32, tag="rec")
                            nc.scalar.activation(rec[:], lnd[:], AF.Exp,
                                                 scale=-1.0)
                            bc = tp2.tile([64, TPC], dt.float32, tag="bc")
                            nc.gpsimd.partition_broadcast(bc[:], rec[:])
                            nc.vector.tensor_mul(
                                ot_v[hp:hp + DH, hc, :], pot[0:DH, :], bc[:])
                dump("ot", otsb[:], l)

                # ---- wo + residual
                with nc.named_scope(f"L{l}_wo"):
                    wov = wo[l].rearrange("(c p) o -> p c o", p=128)

                    def wo_consume(ps, jj):
                        nc.vector.tensor_add(x_v[:, jj, :], x_v[:, jj, :], ps[:])
                    stream_mm_dmajor(wov, 0, DIM, ot_v, wo_consume)
                dump("xattn", x_sb[:], l)

                with nc.named_scope(f"L{l}_ln2"):
                    layer_norm(2 * l + 1, x_v, xln_v)

                # ---- FFN
                with nc.named_scope(f"L{l}_ffn1"):
                    nc.sync.dma_start(
                        b1sb[:], b1[l].rearrange("(f p) -> p f", p=128))
                    w1v = w1[l].rearrange("(c p) o -> p c o", p=128)

                    def ffn1_consume(ps, f):
                        nc.scalar.activation(
                            h1_v[:, f, :], ps[:], AF.Gelu,
                            bias=b1sb[:, f:f + 1])
                    stream_mm_dmajor(w1v, 0, FF, xln_v, ffn1_consume)
                dump("h1", h1sb[:], l)

                nc.sync.dma_start(
                    b2sb[:], b2[l].rearrange("(c p) -> p c", p=128))
                w2v = w2[l].rearrange("(f p) o -> p f o", p=128)

                with nc.named_scope(f"L{l}_ffn2"):
                    for j in range(DC):
                        ps = ppo.tile([128, TPC], dt.float32, tag="attno")
                        for half in range(2):
                            wt = wp.tile([128, (FC // 2) * 128], dt.bfloat16,
                                         tag="w")
                            nc.sync.dma_start(
                                wt[:],
                                w2v[:, (FC // 2) * half:(FC // 2) * (half + 1),
                                    128 * j:128 * (j + 1)])
                            wt_v = wt.rearrange("p (f o) -> p f o", f=FC // 2)
                            for f2 in range(FC // 2):
                                f = (FC // 2) * half + f2
                                nc.tensor.matmul(
                                    ps[:], wt_v[:, f2, :], h1_v[:, f, :],
                                    start=(f == 0), stop=(f == FC - 1))
                        # x += psum + b2  (b2 is per-feature = per-partition)
                        t3 = tp2.tile([128, TPC], dt.float32, tag="res2")
                        nc.vector.tensor_scalar_add(
                            t3[:], ps[:], b2sb[:, j:j + 1])
                        nc.vector.tensor_add(x_v[:, j, :], x_v[:, j, :], t3[:])
                dump("xl0", x_sb[:], l)

            nc.sync.dma_start(xO.rearrange("(c p) t -> p c t", p=128),
                              x_v[:, :, :])

    nc.compile()
    return nc


def _prepare(inputs):
    x = np.ascontiguousarray(np.asarray(inputs["x"], dtype=np.float32))
    mask = np.asarray(inputs["mask"])
    wqkv = np.asarray(inputs["wqkv"], dtype=np.float32)
    wo = np.asarray(inputs["wo"], dtype=np.float32)
    w1 = np.asarray(inputs["w1"], dtype=np.float32)
    w2 = np.asarray(inputs["w2"], dtype=np.float32)
    b1 = np.asarray(inputs["b1"], dtype=np.float32)
    b2 = np.asarray(inputs["b2"], dtype=np.float32)
    ln1_g = np.asarray(inputs["ln1_g"], dtype=np.float32)
    ln1_b = np.asarray(inputs["ln1_b"], dtype=np.float32)
    ln2_g = np.asarray(inputs["ln2_g"], dtype=np.float32)
    ln2_b = np.asarray(inputs["ln2_b"], dtype=np.float32)

    ln_affine = not (
        np.all(ln1_g == 1.0) and np.all(ln1_b == 0.0)
        and np.all(ln2_g == 1.0) and np.all(ln2_b == 0.0))
    mask_b = np.asarray(mask, bool)
    masked = not np.all(mask_b)

    key = (ln_affine, masked)
    if key not in _CACHE:
        _CACHE[key] = _build(ln_affine, masked)
    nc = _CACHE[key]

    wqkv_bf = np.ascontiguousarray(wqkv.astype(BF16))
    wo_bf = np.ascontiguousarray(wo.astype(BF16))
    w1_bf = np.ascontiguousarray(w1.astype(BF16))
    w2_bf = np.ascontiguousarray(w2.astype(BF16))
    amult = mask_b.astype(np.float32)   # 1 visible / 0 masked

    in_maps = []
    for c in range(N_CORES):
        b = c // GROUP
        s = c % GROUP
        xT = np.ascontiguousarray(x[b, s * TPC:(s + 1) * TPC, :].T)
        m = {
            "xT": xT, "wqkv": wqkv_bf, "wo": wo_bf, "w1": w1_bf, "w2": w2_bf,
            "b1": b1, "b2": b2,
        }
        if masked:
            # permute keys to gathered chunk order: ci = h*8 + r*2 + g
            am = np.empty(SEQ, np.float32)
            ci = 0
            for h in range(2):
                for r in range(GROUP):
                    for g in range(HT // 128):
                        src = r * TPC + h * HT + g * 128
                        am[ci * 128:(ci + 1) * 128] = amult[b, src:src + 128]
                        ci += 1
            m["amult"] = np.ascontiguousarray(am)
        if ln_affine:
            g = np.stack([v for pair in zip(ln1_g, ln2_g) for v in pair])
            bb = np.stack([v for pair in zip(ln1_b, ln2_b) for v in pair])
            m["ln_g"] = np.ascontiguousarray(g)
            m["ln_b"] = np.ascontiguousarray(bb)
        in_maps.append(m)
    return nc, in_maps


def _assemble(results):
    out = np.empty((BATCH, SEQ, DIM), np.float32)
    for c in range(N_CORES):
        b = c // GROUP
        s = c % GROUP
        out[b, s * TPC:(s + 1) * TPC, :] = results[c]["xO"].T
    return out


def kernel(**inputs):
    nc, in_maps = _prepare(inputs)
    res = run_bass_kernel_spmd(nc, in_maps, list(range(N_CORES)))
    return _assemble(res.results)


def run_traced(**inputs):
    """Used by test.py only: run with NTFF profiling enabled."""
    nc, in_maps = _prepare(inputs)
    return run_bass_kernel_spmd(nc, in_maps, list(range(N_CORES)), trace=True)


# revision 15
# speedup vs baseline: 1.1329x; 1.0284x over previous
"""Trainium2 Bass kernel for a 4-layer transformer encoder.

Sharding: 8-way data-parallel over tokens. Cores 0-3 handle batch element 0,
cores 4-7 batch element 1; each core owns a 512-token slice of its batch
element's 2048-token sequence. Attention needs full-sequence K/V, so each
layer AllGathers the core's K/V slice within its 4-core group — split into
two token-half collectives so the wire time overlaps KV/Q compute and the
gathered loads pipeline.

On-chip layout: activations are feature-major ([d, tokens]); LayerNorm
statistics are computed with ones-vector matmuls (partition-axis reduction),
softmax runs on transposed scores ([keys, queries]) so the denominator comes
from an appended ones-column in V and no transposes are needed anywhere.
Attention processes heads in pairs: the two 64-row score matmuls of a pair
run concurrently in disjoint PE row-groups (tile_position packing), which
keeps the full 128x128 array active. Matmuls run in bf16 with fp32 PSUM
accumulation; the residual stream stays fp32 in SBUF.
"""
import numpy as np
import ml_dtypes

import concourse.bass as bass
import concourse.bacc as bacc
import concourse.tile as tile
import concourse.mybir as mybir
from concourse.bass_utils import run_bass_kernel_spmd

dt = mybir.dt
AF = mybir.ActivationFunctionType
OP = mybir.AluOpType
BF16 = ml_dtypes.bfloat16

DIM = 1024
DEPTH = 4
HEADS = 16
DH = 64
INNER = 1024
FF = 4096
EPS = 1e-5
SEQ = 2048
BATCH = 2
N_CORES = 8
GROUP = 4                 # cores per batch element
TPC = SEQ // GROUP        # tokens per core = 512
HT = TPC // 2             # tokens per AG half = 256
DC = DIM // 128           # 8 dmodel chunks
KC = SEQ // 128           # 16 key chunks
FC = FF // 128            # 32 ff chunks
VW = HEADS * (DH + 1)     # v tile row width per key chunk (ones col appended)
KV_H = DIM * HT           # elements in one k (or v) half-slice
WSL = 256                 # weight slice column width

_CACHE = {}


class _Bacc(bacc.Bacc):
    """Bacc with activation-table thrash fix: restrict the table-set choices
    so Exp/Ln resolve to natural_log_exp_and_others and Gelu to
    gelu_and_others (set order/ids preserved; only contents filtered)."""

    _KEEP = {"natural_log_exp_and_others", "gelu_and_others"}

    def insert_act_table_loads(self):
        from concourse.hw_specs import get_activation_tables
        import bass_rust as _bass_rust

        has_activation = any(
            isinstance(i, mybir.InstActivation)
            for b in self.main_func.blocks
            for i in b.instructions
        )
        if not has_activation:
            return
        tables = [
            (name, fns if name in self._KEEP else set())
            for name, fns in get_activation_tables(self.m.arch).items()
        ]
        _bass_rust.insert_act_table_loads(self, tables)


DBG_SHAPES = {
    "xln": ([128, DC * TPC], dt.bfloat16),
    "q": ([128, DC * TPC], dt.bfloat16),
    "ksb": ([128, DC * SEQ], dt.bfloat16),
    "vsb": ([128, KC * VW], dt.bfloat16),
    "ot": ([128, DC * TPC], dt.bfloat16),
    "xattn": ([128, DC * TPC], dt.float32),
    "h1": ([128, FC * TPC], dt.bfloat16),
    "xl0": ([128, DC * TPC], dt.float32),
}


def _build(ln_affine: bool, masked: bool = False, dbg: str | None = None):
    nc = _Bacc("TRN2", target_bir_lowering=False, debug=False,
               num_devices=N_CORES)

    xT = nc.dram_tensor("xT", [DIM, TPC], dt.float32, kind="ExternalInput").ap()
    wqkv = nc.dram_tensor("wqkv", [DEPTH, DIM, 3 * INNER], dt.bfloat16,
                          kind="ExternalInput").ap()
    wo = nc.dram_tensor("wo", [DEPTH, INNER, DIM], dt.bfloat16,
                        kind="ExternalInput").ap()
    w1 = nc.dram_tensor("w1", [DEPTH, DIM, FF], dt.bfloat16,
                        kind="ExternalInput").ap()
    w2 = nc.dram_tensor("w2", [DEPTH, FF, DIM], dt.bfloat16,
                        kind="ExternalInput").ap()
    b1 = nc.dram_tensor("b1", [DEPTH, FF], dt.float32,
                        kind="ExternalInput").ap()
    b2 = nc.dram_tensor("b2", [DEPTH, DIM], dt.float32,
                        kind="ExternalInput").ap()
    if masked:
        # multiplicative key mask (1 visible / 0 masked), applied to V rows;
        # host pre-permutes to the gathered key-chunk order.
        amult = nc.dram_tensor("amult", [SEQ], dt.float32,
                               kind="ExternalInput").ap()
    if ln_affine:
        ln_g = nc.dram_tensor("ln_g", [2 * DEPTH, DIM], dt.float32,
                              kind="ExternalInput").ap()
        ln_b = nc.dram_tensor("ln_b", [2 * DEPTH, DIM], dt.float32,
                              kind="ExternalInput").ap()
    xO = nc.dram_tensor("xO", [DIM, TPC], dt.float32,
                        kind="ExternalOutput").ap()
    dbg_t = None
    if dbg is not None:
        shp, ddt = DBG_SHAPES[dbg]
        dbg_t = nc.dram_tensor("dbg", shp, ddt, kind="ExternalOutput").ap()

    groups = [[0, 1, 2, 3], [4, 5, 6, 7]]

    with tile.TileContext(nc) as tc:
        with (
            tc.tile_pool(name="pers", bufs=1) as pers,
            tc.tile_pool(name="wp", bufs=4) as wp,
            tc.tile_pool(name="tp2", bufs=2) as tp2,
            tc.tile_pool(name="tp3", bufs=3) as tp3,
            tc.tile_pool(name="es", bufs=4) as esp,
            tc.tile_pool(name="st", bufs=2) as stp,
            tc.tile_pool(name="pp", bufs=2, space="PSUM") as pp,
            tc.tile_pool(name="ppo", bufs=2, space="PSUM") as ppo,
            tc.tile_pool(name="pst", bufs=2, space="PSUM") as pst,
            tc.tile_pool(name="dram", bufs=2, space="DRAM") as dram,
        ):
            # ---- persistent tiles
            x_sb = pers.tile([128, DC * TPC], dt.float32)     # residual, d-major
            xln = pers.tile([128, DC * TPC], dt.bfloat16)     # ln output, d-major
            qbf = pers.tile([128, DC * TPC], dt.bfloat16)     # q, d-major
            ksb = pers.tile([128, DC * SEQ], dt.bfloat16)     # gathered k, d-major
            vsb = pers.tile([128, KC * VW], dt.bfloat16)      # gathered v + ones
            otsb = pers.tile([128, DC * TPC], dt.bfloat16)    # attn out^T, d-major
            h1sb = pers.tile([128, FC * TPC], dt.bfloat16)    # ffn hidden
            onesb = pers.tile([128, 1], dt.bfloat16)
            epsb = pers.tile([1, 1], dt.float32)
            b1sb = pers.tile([128, FC], dt.float32)
            b2sb = pers.tile([128, DC], dt.float32)
            absb = pers.tile([128, KC], dt.float32) if masked else None
            if ln_affine:
                lngsb = pers.tile([128, 2 * DEPTH * DC], dt.float32)
                lnbsb = pers.tile([128, 2 * DEPTH * DC], dt.float32)

            x_v = x_sb.rearrange("p (c t) -> p c t", c=DC)
            xln_v = xln.rearrange("p (c t) -> p c t", c=DC)
            q_v = qbf.rearrange("p (c t) -> p c t", c=DC)
            k_v = ksb.rearrange("p (c t) -> p c t", c=DC)
            vs_v = vsb.rearrange("p (g h e) -> p g h e", g=KC, h=HEADS, e=DH + 1)
            ot_v = otsb.rearrange("p (c t) -> p c t", c=DC)
            h1_v = h1sb.rearrange("p (f t) -> p f t", f=FC)

            nc.vector.memset(onesb[:], 1.0)
            nc.vector.memset(epsb[:], EPS)
            nc.gpsimd.memset(vs_v[:, :, :, DH:DH + 1], 1.0)
            nc.sync.dma_start(x_sb[:], xT.rearrange("(c p) t -> p c t", p=128))
            if masked:
                nc.sync.dma_start(
                    absb[:], amult.rearrange("(k p) -> p k", p=128))
            if ln_affine:
                nc.sync.dma_start(
                    lngsb[:], ln_g.rearrange("l (c p) -> p (l c)", p=128))
                nc.sync.dma_start(
                    lnbsb[:], ln_b.rearrange("l (c p) -> p (l c)", p=128))

            def layer_norm(l2, src_v, dst_v, t0=0, t1=TPC):
                """dst (bf16) = layernorm(src) along partition-major feature
                dim, for token range [t0, t1)."""
                W = t1 - t0
                psum_s = pst.tile([1, TPC], dt.float32, tag="lnps")
                psum_q = pst.tile([1, TPC], dt.float32, tag="lnps")
                for c in range(DC):
                    xb = tp3.tile([128, TPC], dt.bfloat16, tag="lncast")
                    nc.vector.tensor_copy(xb[:, 0:W], src_v[:, c, t0:t1])
                    x2 = tp3.tile([128, TPC], dt.bfloat16, tag="lnsq")
                    nc.vector.tensor_mul(x2[:, 0:W], xb[:, 0:W], xb[:, 0:W])
                    nc.tensor.matmul(psum_s[0:1, 0:W], onesb[:], xb[:, 0:W],
                                     start=(c == 0), stop=(c == DC - 1))
                    nc.tensor.matmul(psum_q[0:1, 0:W], onesb[:], x2[:, 0:W],
                                     start=(c == 0), stop=(c == DC - 1))
                mu = stp.tile([1, TPC], dt.float32, tag="stat")
                nc.vector.tensor_scalar_mul(mu[0:1, 0:W], psum_s[0:1, 0:W],
                                            1.0 / DIM)
                mub = tp2.tile([128, TPC], dt.float32, tag="mub")
                nc.gpsimd.partition_broadcast(mub[:, 0:W], mu[0:1, 0:W])
                negmusq = stp.tile([1, TPC], dt.float32, tag="stat")
                nc.vector.scalar_tensor_tensor(
                    out=negmusq[0:1, 0:W], in0=mu[0:1, 0:W], scalar=-1.0,
                    in1=mu[0:1, 0:W], op0=OP.mult, op1=OP.mult)
                var = stp.tile([1, TPC], dt.float32, tag="stat")
                nc.vector.scalar_tensor_tensor(
                    out=var[0:1, 0:W], in0=psum_q[0:1, 0:W], scalar=1.0 / DIM,
                    in1=negmusq[0:1, 0:W], op0=OP.mult, op1=OP.add)
                lnv = stp.tile([1, TPC], dt.float32, tag="stat")
                nc.scalar.activation(lnv[0:1, 0:W], var[0:1, 0:W], AF.Ln,
                                     bias=epsb[:])
                rstd = stp.tile([1, TPC], dt.float32, tag="stat")
                nc.scalar.activation(rstd[0:1, 0:W], lnv[0:1, 0:W], AF.Exp,
                                     scale=-0.5)
                rsb = tp2.tile([128, TPC], dt.float32, tag="rsb")
                nc.gpsimd.partition_broadcast(rsb[:, 0:W], rstd[0:1, 0:W])
                for c in range(DC):
                    t1t = tp3.tile([128, TPC], dt.float32, tag="lnt1")
                    nc.vector.tensor_sub(t1t[:, 0:W], src_v[:, c, t0:t1],
                                         mub[:, 0:W])
                    if ln_affine:
                        t2 = tp3.tile([128, TPC], dt.float32, tag="lnt2")
                        nc.vector.tensor_mul(t2[:, 0:W], t1t[:, 0:W],
                                             rsb[:, 0:W])
                        nc.vector.tensor_scalar(
                            dst_v[:, c, t0:t1], t2[:, 0:W],
                            lngsb[:, l2 * DC + c:l2 * DC + c + 1],
                            lnbsb[:, l2 * DC + c:l2 * DC + c + 1],
                            OP.mult, OP.add)
                    else:
                        nc.vector.tensor_mul(dst_v[:, c, t0:t1], t1t[:, 0:W],
                                             rsb[:, 0:W])

            def stream_mm_dmajor(w_src, col0, ncols, rhs_v, consume):
                """out[cols, tok] = W[:, col0:col0+ncols].T @ act, d-major act as
                rhs. consume(ps, jj) takes psum [128, TPC] for output-col chunk
                jj (128 cols each, numbered from col0/128)."""
                for s in range(ncols // WSL):
                    wt = wp.tile([128, DC * WSL], dt.bfloat16, tag="w")
                    nc.sync.dma_start(
                        wt[:], w_src[:, :, col0 + WSL * s:col0 + WSL * (s + 1)])
                    wt_v = wt.rearrange("p (c o) -> p c o", c=DC)
                    for j in range(WSL // 128):
                        ps = pp.tile([128, TPC], dt.float32, tag="mm")
                        for c in range(DC):
                            nc.tensor.matmul(
                                ps[:], wt_v[:, c, 128 * j:128 * (j + 1)],
                                rhs_v[:, c, :],
                                start=(c == 0), stop=(c == DC - 1))
                        consume(ps, (col0 + WSL * s) // 128 + j)

            def dump(name, src_ap, l):
                if dbg == name and l == 0:
                    nc.sync.dma_start(dbg_t[:], src_ap)

            kvag = {}

            def ln1_kv_half(l, h):
                """ln1 for token half h of layer l, then K/V for that half and
                its AllGather trigger. Emitted inside layer l-1's FFN2 so the
                collective wire rides under compute."""
                if l not in kvag:
                    kvag[l] = (
                        [dram.tile([2 * KV_H], dt.bfloat16,
                                   name=f"kvin{l}h{hh}", tag=f"kvin{hh}")
                         for hh in range(2)],
                        [dram.tile([GROUP * 2 * KV_H], dt.bfloat16,
                                   name=f"kvag{l}h{hh}", tag=f"kvag{hh}")
                         for hh in range(2)])
                kv_in, ag = kvag[l]
                with nc.named_scope(f"L{l}_ln1h{h}"):
                    layer_norm(2 * l, x_v, xln_v, HT * h, HT * (h + 1))
                wq = wqkv[l].rearrange("(c p) o -> p c o", p=128)
                with nc.named_scope(f"L{l}_kv{h}"):
                    k_dst = kv_in[h][0:KV_H].rearrange("(r t) -> r t", t=HT)
                    for s in range(INNER // WSL):
                        wt = wp.tile([128, DC * WSL], dt.bfloat16, tag="w")
                        nc.sync.dma_start(
                            wt[:],
                            wq[:, :, INNER + WSL * s:INNER + WSL * (s + 1)])
                        wt_v = wt.rearrange("p (c o) -> p c o", c=DC)
                        for j in range(WSL // 128):
                            ps = pp.tile([128, TPC], dt.float32, tag="mm")
                            for c in range(DC):
                                nc.tensor.matmul(
                                    ps[:, 0:HT],
                                    wt_v[:, c, 128 * j:128 * (j + 1)],
                                    xln_v[:, c, HT * h:HT * (h + 1)],
                                    start=(c == 0), stop=(c == DC - 1))
                            kown = tp3.tile([128, HT], dt.bfloat16,
                                            tag="kv_own")
                            nc.vector.tensor_copy(kown[:], ps[:, 0:HT])
                            r = 128 * (2 * s + j)
                            nc.scalar.dma_start(k_dst[r:r + 128, :], kown[:])
                    # V for this half: token-major out
                    v_dst = kv_in[h][KV_H:2 * KV_H].rearrange(
                        "(t v) -> t v", v=INNER)
                    for s in range(INNER // WSL):
                        wt = wp.tile([128, DC * WSL], dt.bfloat16, tag="w")
                        nc.sync.dma_start(
                            wt[:],
                            wq[:, :, 2 * INNER + WSL * s:
                               2 * INNER + WSL * (s + 1)])
                        wt_v = wt.rearrange("p (c o) -> p c o", c=DC)
                        for t in range(2):
                            ps = pp.tile([128, TPC], dt.float32, tag="mm")
                            for c in range(DC):
                                nc.tensor.matmul(
                                    ps[:, 0:WSL],
                                    xln_v[:, c,
                                          HT * h + 128 * t:
                                          HT * h + 128 * (t + 1)],
                                    wt_v[:, c, :],
                                    start=(c == 0), stop=(c == DC - 1))
                            vown = tp3.tile([128, WSL], dt.bfloat16,
                                            tag="v_own")
                            nc.vector.tensor_copy(vown[:], ps[:, 0:WSL])
                            nc.scalar.dma_start(
                                v_dst[128 * t:128 * (t + 1),
                                      WSL * s:WSL * (s + 1)], vown[:])
                    nc.gpsimd.collective_compute(
                        "AllGather", OP.bypass,
                        ins=[kv_in[h].opt()], outs=[ag[h].opt()],
                        replica_groups=groups)

            for l in range(DEPTH):
                for h in range(2):
                    ln1_kv_half(l, h)
                kv_in, ag = kvag[l]
                wq = wqkv[l].rearrange("(c p) o -> p c o", p=128)
                dump("xln", xln[:], l)

                # ---- q for own tokens (overlaps the AllGathers)
                with nc.named_scope(f"L{l}_q"):
                    def q_consume(ps, jj):
                        nc.vector.tensor_copy(q_v[:, jj, :], ps[:])
                    stream_mm_dmajor(wq, 0, INNER, xln_v, q_consume)
                dump("q", qbf[:], l)

                # ---- load gathered k, v; key chunk ci = h*8 + r*2 + g
                with nc.named_scope(f"L{l}_kvload"):
                    for h in range(2):
                        for r in range(GROUP):
                            base = r * 2 * KV_H
                            off = (h * GROUP + r) * HT
                            k_src = ag[h][base:base + KV_H].rearrange(
                                "(c p t) -> p c t", p=128, t=HT)
                            nc.gpsimd.dma_start(
                                k_v[:, :, off:off + HT], k_src)
                            v_src = ag[h][base + KV_H:base + 2 * KV_H].rearrange(
                                "(g p hh e) -> p g hh e", p=128, hh=HEADS, e=DH)
                            for g in range(HT // 128):
                                nc.gpsimd.dma_start(
                                    vs_v[:, off // 128 + g, :, 0:DH],
                                    v_src[:, g])
                    if masked:
                        vv = vsb.rearrange("p (g w) -> p g w", g=KC)
                        for kc in range(KC):
                            nc.vector.tensor_scalar_mul(
                                vv[:, kc, :], vv[:, kc, :],
                                absb[:, kc:kc + 1])
                dump("ksb", ksb[:], l)
                dump("vsb", vsb[:], l)

                # ---- attention, head-pair at a time; the two 64-row score
                # matmuls of a pair run concurrently in disjoint PE row groups.
                with nc.named_scope(f"L{l}_attn"):
                    for hc in range(HEADS // 2):
                        h0, h1 = 2 * hc, 2 * hc + 1
                        po_a = ppo.tile([128, TPC], dt.float32, tag="attno")
                        po_b = ppo.tile([128, TPC], dt.float32, tag="attno")
                        es_q = [None] * KC

                        def attn_v(kc):
                            es2 = es_q[kc]
                            nc.tensor.matmul(
                                po_a[0:DH + 1, :],
                                vsb[:, kc * VW + h0 * (DH + 1):
                                    kc * VW + (h0 + 1) * (DH + 1)],
                                es2[:, 0:TPC],
                                start=(kc == 0), stop=(kc == KC - 1))
                            nc.tensor.matmul(
                                po_b[0:DH + 1, :],
                                vsb[:, kc * VW + h1 * (DH + 1):
                                    kc * VW + (h1 + 1) * (DH + 1)],
                                es2[:, TPC:2 * TPC],
                                start=(kc == 0), stop=(kc == KC - 1))

                        # skew-2 software pipeline: attnV(kc-2) issues after
                        # scores(kc) so the PE FIFO never waits on exp.
                        for kc in range(KC):
                            if kc >= 2:
                                attn_v(kc - 2)
                            ps2 = pp.tile([128, 2 * TPC], dt.float32, tag="mm")
                            nc.tensor.matmul(
                                ps2[:, 0:TPC],
                                k_v[0:DH, hc, 128 * kc:128 * (kc + 1)],
                                q_v[0:DH, hc, :],
                                start=True, stop=True, tile_position=(0, 0))
                            nc.tensor.matmul(
                                ps2[:, TPC:2 * TPC],
                                k_v[DH:128, hc, 128 * kc:128 * (kc + 1)],
                                q_v[DH:128, hc, :],
                                start=True, stop=True, tile_position=(64, 0))
                            es2 = esp.tile([128, 2 * TPC], dt.bfloat16,
                                           tag="es")
                            nc.scalar.activation(es2[:], ps2[:], AF.Exp,
                                                 scale=DH ** -0.5)
                            es_q[kc] = es2
                        attn_v(KC - 2)
                        attn_v(KC - 1)
                        # evacuate PSUM promptly (frees po for the next pair),
                        # then normalize from SBUF off the critical path.
                        for po, h in ((po_a, h0), (po_b, h1)):
                            hp = 64 * (h % 2)
                            pot = tp2.tile([DH + 1, TPC], dt.float32,
                                           tag="pot")
                            nc.vector.tensor_copy(pot[:], po[0:DH + 1, :])
                            # reciprocal on DVE: keeps the denominator chain
                            # off the exp-saturated ACT queue
                            rec = stp.tile([1, TPC], dt.float32, tag="rec")
                            nc.vector.reciprocal(rec[:], pot[DH:DH + 1, :])
                            bc = tp2.tile([64, TPC], dt.float32, tag="bc")
                            nc.gpsimd.partition_broadcast(bc[:], rec[:])
                            nc.vector.tensor_mul(
                                ot_v[hp:hp + DH, hc, :], pot[0:DH, :], bc[:])
                dump("ot", otsb[:], l)

                # ---- wo + residual
                with nc.named_scope(f"L{l}_wo"):
                    wov = wo[l].rearrange("(c p) o -> p c o", p=128)

                    def wo_consume(ps, jj):
                        nc.vector.tensor_add(x_v[:, jj, :], x_v[:, jj, :], ps[:])
                    stream_mm_dmajor(wov, 0, DIM, ot_v, wo_consume)
                dump("xattn", x_sb[:], l)

                with nc.named_scope(f"L{l}_ln2"):
                    layer_norm(2 * l + 1, x_v, xln_v)

                # ---- FFN
                with nc.named_scope(f"L{l}_ffn1"):
                    nc.sync.dma_start(
                        b1sb[:], b1[l].rearrange("(f p) -> p f", p=128))
                    w1v = w1[l].rearrange("(c p) o -> p c o", p=128)

                    def ffn1_consume(ps, f):
                        nc.scalar.activation(
                            h1_v[:, f, :], ps[:], AF.Gelu,
                            bias=b1sb[:, f:f + 1])
                    stream_mm_dmajor(w1v, 0, FF, xln_v, ffn1_consume)
                dump("h1", h1sb[:], l)

                nc.sync.dma_start(
                    b2sb[:], b2[l].rearrange("(c p) -> p c", p=128))
                w2v = w2[l].rearrange("(f p) o -> p f o", p=128)

                with nc.named_scope(f"L{l}_ffn2"):
                    for j in range(DC):
                        ps = ppo.tile([128, TPC], dt.float32, tag="attno")
                        for half in range(2):
                            wt = wp.tile([128, (FC // 2) * 128], dt.bfloat16,
                                         tag="w")
                            nc.sync.dma_start(
                                wt[:],
                                w2v[:, (FC // 2) * half:(FC // 2) * (half + 1),
                                    128 * j:128 * (j + 1)])
                            wt_v = wt.rearrange("p (f o) -> p f o", f=FC // 2)
                            for f2 in range(FC // 2):
                                f = (FC // 2) * half + f2
                                nc.tensor.matmul(
                                    ps[:], wt_v[:, f2, :], h1_v[:, f, :],
                                    start=(f == 0), stop=(f == FC - 1))
                        # x += psum + b2  (b2 is per-feature = per-partition)
                        t3 = tp2.tile([128, TPC], dt.float32, tag="res2")
                        nc.vector.tensor_scalar_add(
                            t3[:], ps[:], b2sb[:, j:j + 1])
                        nc.vector.tensor_add(x_v[:, j, :], x_v[:, j, :], t3[:])
                dump("xl0", x_sb[:], l)

            nc.sync.dma_start(xO.rearrange("(c p) t -> p c t", p=128),
                              x_v[:, :, :])

    nc.compile()
    return nc


def _prepare(inputs):
    x = np.ascontiguousarray(np.asarray(inputs["x"], dtype=np.float32))
    mask = np.asarray(inputs["mask"])
    wqkv = np.asarray(inputs["wqkv"], dtype=np.float32)
    wo = np.asarray(inputs["wo"], dtype=np.float32)
    w1 = np.asarray(inputs["w1"], dtype=np.float32)
    w2 = np.asarray(inputs["w2"], dtype=np.float32)
    b1 = np.asarray(inputs["b1"], dtype=np.float32)
    b2 = np.asarray(inputs["b2"], dtype=np.float32)
    ln1_g = np.asarray(inputs["ln1_g"], dtype=np.float32)
    ln1_b = np.asarray(inputs["ln1_b"], dtype=np.float32)
    ln2_g = np.asarray(inputs["ln2_g"], dtype=np.float32)
    ln2_b = np.asarray(inputs["ln2_b"], dtype=np.float32)

    ln_affine = not (
        np.all(ln1_g == 1.0) and np.all(ln1_b == 0.0)
        and np.all(ln2_g == 1.0) and np.all(ln2_b == 0.0))
    mask_b = np.asarray(mask, bool)
    masked = not np.all(mask_b)

    key = (ln_affine, masked)
    if key not in _CACHE:
        _CACHE[key] = _build(ln_affine, masked)
    nc = _CACHE[key]

    wqkv_bf = np.ascontiguousarray(wqkv.astype(BF16))
    wo_bf = np.ascontiguousarray(wo.astype(BF16))
    w1_bf = np.ascontiguousarray(w1.astype(BF16))
    w2_bf = np.ascontiguousarray(w2.astype(BF16))
    amult = mask_b.astype(np.float32)   # 1 visible / 0 masked

    in_maps = []
    for c in range(N_CORES):
        b = c // GROUP
        s = c % GROUP
        xT = np.ascontiguousarray(x[b, s * TPC:(s + 1) * TPC, :].T)
        m = {
            "xT": xT, "wqkv": wqkv_bf, "wo": wo_bf, "w1": w1_bf, "w2": w2_bf,
            "b1": b1, "b2": b2,
        }
        if masked:
            # permute keys to gathered chunk order: ci = h*8 + r*2 + g
            am = np.empty(SEQ, np.float32)
            ci = 0
            for h in range(2):
                for r in range(GROUP):
                    for g in range(HT // 128):
                        src = r * TPC + h * HT + g * 128
                        am[ci * 128:(ci + 1) * 128] = amult[b, src:src + 128]
                        ci += 1
            m["amult"] = np.ascontiguousarray(am)
        if ln_affine:
            g = np.stack([v for pair in zip(ln1_g, ln2_g) for v in pair])
            bb = np.stack([v for pair in zip(ln1_b, ln2_b) for v in pair])
            m["ln_g"] = np.ascontiguousarray(g)
            m["ln_b"] = np.ascontiguousarray(bb)
        in_maps.append(m)
    return nc, in_maps


def _assemble(results):
    out = np.empty((BATCH, SEQ, DIM), np.float32)
    for c in range(N_CORES):
        b = c // GROUP
        s = c % GROUP
        out[b, s * TPC:(s + 1) * TPC, :] = results[c]["xO"].T
    return out


def kernel(**inputs):
    nc, in_maps = _prepare(inputs)
    res = run_bass_kernel_spmd(nc, in_maps, list(range(N_CORES)))
    return _assemble(res.results)


def run_traced(**inputs):
    """Used by test.py only: run with NTFF profiling enabled."""
    nc, in_maps = _prepare(inputs)
    return run_bass_kernel_spmd(nc, in_maps, list(range(N_CORES)), trace=True)


# revision 17
# speedup vs baseline: 1.1346x; 1.0015x over previous
"""Trainium2 Bass kernel for a 4-layer transformer encoder.

Sharding: 8-way data-parallel over tokens. Cores 0-3 handle batch element 0,
cores 4-7 batch element 1; each core owns a 512-token slice of its batch
element's 2048-token sequence. Attention needs full-sequence K/V, so each
layer AllGathers the core's K/V slice within its 4-core group — split into
two token-half collectives so the wire time overlaps KV/Q compute and the
gathered loads pipeline.

On-chip layout: activations are feature-major ([d, tokens]); LayerNorm
statistics are computed with ones-vector matmuls (partition-axis reduction),
softmax runs on transposed scores ([keys, queries]) so the denominator comes
from an appended ones-column in V and no transposes are needed anywhere.
Attention processes heads in pairs: the two 64-row score matmuls of a pair
run concurrently in disjoint PE row-groups (tile_position packing), which
keeps the full 128x128 array active. Matmuls run in bf16 with fp32 PSUM
accumulation; the residual stream stays fp32 in SBUF.
"""
import numpy as np
import ml_dtypes

import concourse.bass as bass
import concourse.bacc as bacc
import concourse.tile as tile
import concourse.mybir as mybir
from concourse.bass_utils import run_bass_kernel_spmd

dt = mybir.dt
AF = mybir.ActivationFunctionType
OP = mybir.AluOpType
BF16 = ml_dtypes.bfloat16

DIM = 1024
DEPTH = 4
HEADS = 16
DH = 64
INNER = 1024
FF = 4096
EPS = 1e-5
SEQ = 2048
BATCH = 2
N_CORES = 8
GROUP = 4                 # cores per batch element
TPC = SEQ // GROUP        # tokens per core = 512
HT = TPC // 2             # tokens per AG half = 256
DC = DIM // 128           # 8 dmodel chunks
KC = SEQ // 128           # 16 key chunks
FC = FF // 128            # 32 ff chunks
VW = HEADS * (DH + 1)     # v tile row width per key chunk (ones col appended)
KV_H = DIM * HT           # elements in one k (or v) half-slice
WSL = 256                 # weight slice column width

_CACHE = {}


class _Bacc(bacc.Bacc):
    """Bacc with activation-table thrash fix: restrict the table-set choices
    so Exp/Ln resolve to natural_log_exp_and_others and Gelu to
    gelu_and_others (set order/ids preserved; only contents filtered)."""

    _KEEP = {"natural_log_exp_and_others", "gelu_and_others"}

    def insert_act_table_loads(self):
        from concourse.hw_specs import get_activation_tables
        import bass_rust as _bass_rust

        has_activation = any(
            isinstance(i, mybir.InstActivation)
            for b in self.main_func.blocks
            for i in b.instructions
        )
        if not has_activation:
            return
        tables = [
            (name, fns if name in self._KEEP else set())
            for name, fns in get_activation_tables(self.m.arch).items()
        ]
        _bass_rust.insert_act_table_loads(self, tables)


DBG_SHAPES = {
    "xln": ([128, DC * TPC], dt.bfloat16),
    "q": ([128, DC * TPC], dt.bfloat16),
    "ksb": ([128, DC * SEQ], dt.bfloat16),
    "vsb": ([128, KC * VW], dt.bfloat16),
    "ot": ([128, DC * TPC], dt.bfloat16),
    "xattn": ([128, DC * TPC], dt.float32),
    "h1": ([128, FC * TPC], dt.bfloat16),
    "xl0": ([128, DC * TPC], dt.float32),
}


def _build(ln_affine: bool, masked: bool = False, dbg: str | None = None):
    nc = _Bacc("TRN2", target_bir_lowering=False, debug=False,
               num_devices=N_CORES)

    xT = nc.dram_tensor("xT", [DIM, TPC], dt.float32, kind="ExternalInput").ap()
    wqkv = nc.dram_tensor("wqkv", [DEPTH, DIM, 3 * INNER], dt.bfloat16,
                          kind="ExternalInput").ap()
    wo = nc.dram_tensor("wo", [DEPTH, INNER, DIM], dt.bfloat16,
                        kind="ExternalInput").ap()
    w1 = nc.dram_tensor("w1", [DEPTH, DIM, FF], dt.bfloat16,
                        kind="ExternalInput").ap()
    w2 = nc.dram_tensor("w2", [DEPTH, FF, DIM], dt.bfloat16,
                        kind="ExternalInput").ap()
    b1 = nc.dram_tensor("b1", [DEPTH, FF], dt.float32,
                        kind="ExternalInput").ap()
    b2 = nc.dram_tensor("b2", [DEPTH, DIM], dt.float32,
                        kind="ExternalInput").ap()
    if masked:
        # multiplicative key mask (1 visible / 0 masked), applied to V rows;
        # host pre-permutes to the gathered key-chunk order.
        amult = nc.dram_tensor("amult", [SEQ], dt.float32,
                               kind="ExternalInput").ap()
    if ln_affine:
        ln_g = nc.dram_tensor("ln_g", [2 * DEPTH, DIM], dt.float32,
                              kind="ExternalInput").ap()
        ln_b = nc.dram_tensor("ln_b", [2 * DEPTH, DIM], dt.float32,
                              kind="ExternalInput").ap()
    xO = nc.dram_tensor("xO", [DIM, TPC], dt.float32,
                        kind="ExternalOutput").ap()
    dbg_t = None
    if dbg is not None:
        shp, ddt = DBG_SHAPES[dbg]
        dbg_t = nc.dram_tensor("dbg", shp, ddt, kind="ExternalOutput").ap()

    groups = [[0, 1, 2, 3], [4, 5, 6, 7]]

    with tile.TileContext(nc) as tc:
        with (
            tc.tile_pool(name="pers", bufs=1) as pers,
            tc.tile_pool(name="wp", bufs=4) as wp,
            tc.tile_pool(name="tp2", bufs=2) as tp2,
            tc.tile_pool(name="tp3", bufs=3) as tp3,
            tc.tile_pool(name="es", bufs=4) as esp,
            tc.tile_pool(name="st", bufs=2) as stp,
            tc.tile_pool(name="pp", bufs=2, space="PSUM") as pp,
            tc.tile_pool(name="ppo", bufs=2, space="PSUM") as ppo,
            tc.tile_pool(name="pst", bufs=2, space="PSUM") as pst,
            tc.tile_pool(name="dram", bufs=2, space="DRAM") as dram,
        ):
            # ---- persistent tiles
            x_sb = pers.tile([128, DC * TPC], dt.float32)     # residual, d-major
            xln = pers.tile([128, DC * TPC], dt.bfloat16)     # ln output, d-major
            qbf = pers.tile([128, DC * TPC], dt.bfloat16)     # q, d-major
            ksb = pers.tile([128, DC * SEQ], dt.bfloat16)     # gathered k, d-major
            vsb = pers.tile([128, KC * VW], dt.bfloat16)      # gathered v + ones
            otsb = pers.tile([128, DC * TPC], dt.bfloat16)    # attn out^T, d-major
            h1sb = pers.tile([128, FC * TPC], dt.bfloat16)    # ffn hidden
            onesb = pers.tile([128, 1], dt.bfloat16)
            epsb = pers.tile([1, 1], dt.float32)
            b1sb = pers.tile([128, FC], dt.float32)
            b2sb = pers.tile([128, DC], dt.float32)
            absb = pers.tile([128, KC], dt.float32) if masked else None
            if ln_affine:
                lngsb = pers.tile([128, 2 * DEPTH * DC], dt.float32)
                lnbsb = pers.tile([128, 2 * DEPTH * DC], dt.float32)

            x_v = x_sb.rearrange("p (c t) -> p c t", c=DC)
            xln_v = xln.rearrange("p (c t) -> p c t", c=DC)
            q_v = qbf.rearrange("p (c t) -> p c t", c=DC)
            k_v = ksb.rearrange("p (c t) -> p c t", c=DC)
            vs_v = vsb.rearrange("p (g h e) -> p g h e", g=KC, h=HEADS, e=DH + 1)
            ot_v = otsb.rearrange("p (c t) -> p c t", c=DC)
            h1_v = h1sb.rearrange("p (f t) -> p f t", f=FC)

            nc.vector.memset(onesb[:], 1.0)
            nc.vector.memset(epsb[:], EPS)
            nc.gpsimd.memset(vs_v[:, :, :, DH:DH + 1], 1.0)
            nc.sync.dma_start(x_sb[:], xT.rearrange("(c p) t -> p c t", p=128))
            if masked:
                nc.sync.dma_start(
                    absb[:], amult.rearrange("(k p) -> p k", p=128))
            if ln_affine:
                nc.sync.dma_start(
                    lngsb[:], ln_g.rearrange("l (c p) -> p (l c)", p=128))
                nc.sync.dma_start(
                    lnbsb[:], ln_b.rearrange("l (c p) -> p (l c)", p=128))

            def layer_norm(l2, src_v, dst_v, t0=0, t1=TPC):
                """dst (bf16) = layernorm(src) along partition-major feature
                dim, for token range [t0, t1)."""
                W = t1 - t0
                psum_s = pst.tile([1, TPC], dt.float32, tag="lnps")
                psum_q = pst.tile([1, TPC], dt.float32, tag="lnps")
                for c in range(DC):
                    xb = tp3.tile([128, TPC], dt.bfloat16, tag="lncast")
                    nc.vector.tensor_copy(xb[:, 0:W], src_v[:, c, t0:t1])
                    x2 = tp3.tile([128, TPC], dt.bfloat16, tag="lnsq")
                    nc.vector.tensor_mul(x2[:, 0:W], xb[:, 0:W], xb[:, 0:W])
                    nc.tensor.matmul(psum_s[0:1, 0:W], onesb[:], xb[:, 0:W],
                                     start=(c == 0), stop=(c == DC - 1))
                    nc.tensor.matmul(psum_q[0:1, 0:W], onesb[:], x2[:, 0:W],
                                     start=(c == 0), stop=(c == DC - 1))
                mu = stp.tile([1, TPC], dt.float32, tag="stat")
                nc.vector.tensor_scalar_mul(mu[0:1, 0:W], psum_s[0:1, 0:W],
                                            1.0 / DIM)
                mub = tp2.tile([128, TPC], dt.float32, tag="mub")
                nc.gpsimd.partition_broadcast(mub[:, 0:W], mu[0:1, 0:W])
                negmusq = stp.tile([1, TPC], dt.float32, tag="stat")
                nc.vector.scalar_tensor_tensor(
                    out=negmusq[0:1, 0:W], in0=mu[0:1, 0:W], scalar=-1.0,
                    in1=mu[0:1, 0:W], op0=OP.mult, op1=OP.mult)
                var = stp.tile([1, TPC], dt.float32, tag="stat")
                nc.vector.scalar_tensor_tensor(
                    out=var[0:1, 0:W], in0=psum_q[0:1, 0:W], scalar=1.0 / DIM,
                    in1=negmusq[0:1, 0:W], op0=OP.mult, op1=OP.add)
                lnv = stp.tile([1, TPC], dt.float32, tag="stat")
                nc.scalar.activation(lnv[0:1, 0:W], var[0:1, 0:W], AF.Ln,
                                     bias=epsb[:])
                rstd = stp.tile([1, TPC], dt.float32, tag="stat")
                nc.scalar.activation(rstd[0:1, 0:W], lnv[0:1, 0:W], AF.Exp,
                                     scale=-0.5)
                rsb = tp2.tile([128, TPC], dt.float32, tag="rsb")
                nc.gpsimd.partition_broadcast(rsb[:, 0:W], rstd[0:1, 0:W])
                for c in range(DC):
                    t1t = tp3.tile([128, TPC], dt.float32, tag="lnt1")
                    nc.vector.tensor_sub(t1t[:, 0:W], src_v[:, c, t0:t1],
                                         mub[:, 0:W])
                    if ln_affine:
                        t2 = tp3.tile([128, TPC], dt.float32, tag="lnt2")
                        nc.vector.tensor_mul(t2[:, 0:W], t1t[:, 0:W],
                                             rsb[:, 0:W])
                        nc.vector.tensor_scalar(
                            dst_v[:, c, t0:t1], t2[:, 0:W],
                            lngsb[:, l2 * DC + c:l2 * DC + c + 1],
                            lnbsb[:, l2 * DC + c:l2 * DC + c + 1],
                            OP.mult, OP.add)
                    else:
                        nc.vector.tensor_mul(dst_v[:, c, t0:t1], t1t[:, 0:W],
                                             rsb[:, 0:W])

            def stream_mm_dmajor(w_src, col0, ncols, rhs_v, consume):
                """out[cols, tok] = W[:, col0:col0+ncols].T @ act, d-major act as
                rhs. consume(ps, jj) takes psum [128, TPC] for output-col chunk
                jj (128 cols each, numbered from col0/128)."""
                for s in range(ncols // WSL):
                    wt = wp.tile([128, DC * WSL], dt.bfloat16, tag="w")
                    nc.sync.dma_start(
                        wt[:], w_src[:, :, col0 + WSL * s:col0 + WSL * (s + 1)])
                    wt_v = wt.rearrange("p (c o) -> p c o", c=DC)
                    for j in range(WSL // 128):
                        ps = pp.tile([128, TPC], dt.float32, tag="mm")
                        for c in range(DC):
                            nc.tensor.matmul(
                                ps[:], wt_v[:, c, 128 * j:128 * (j + 1)],
                                rhs_v[:, c, :],
                                start=(c == 0), stop=(c == DC - 1))
                        consume(ps, (col0 + WSL * s) // 128 + j)

            def dump(name, src_ap, l):
                if dbg == name and l == 0:
                    nc.sync.dma_start(dbg_t[:], src_ap)

            kvag = {}

            def ln1_kv_half(l, h):
                """ln1 for token half h of layer l, then K/V for that half and
                its AllGather trigger. Emitted inside layer l-1's FFN2 so the
                collective wire rides under compute."""
                if l not in kvag:
                    kvag[l] = (
                        [dram.tile([2 * KV_H], dt.bfloat16,
                                   name=f"kvin{l}h{hh}", tag=f"kvin{hh}")
                         for hh in range(2)],
                        [dram.tile([GROUP * 2 * KV_H], dt.bfloat16,
                                   name=f"kvag{l}h{hh}", tag=f"kvag{hh}")
                         for hh in range(2)])
                kv_in, ag = kvag[l]
                with nc.named_scope(f"L{l}_ln1h{h}"):
                    layer_norm(2 * l, x_v, xln_v, HT * h, HT * (h + 1))
                wq = wqkv[l].rearrange("(c p) o -> p c o", p=128)
                with nc.named_scope(f"L{l}_kv{h}"):
                    k_dst = kv_in[h][0:KV_H].rearrange("(r t) -> r t", t=HT)
                    for s in range(INNER // WSL):
                        wt = wp.tile([128, DC * WSL], dt.bfloat16, tag="w")
                        nc.sync.dma_start(
                            wt[:],
                            wq[:, :, INNER + WSL * s:INNER + WSL * (s + 1)])
                        wt_v = wt.rearrange("p (c o) -> p c o", c=DC)
                        for j in range(WSL // 128):
                            ps = pp.tile([128, TPC], dt.float32, tag="mm")
                            for c in range(DC):
                                nc.tensor.matmul(
                                    ps[:, 0:HT],
                                    wt_v[:, c, 128 * j:128 * (j + 1)],
                                    xln_v[:, c, HT * h:HT * (h + 1)],
                                    start=(c == 0), stop=(c == DC - 1))
                            kown = tp3.tile([128, HT], dt.bfloat16,
                                            tag="kv_own")
                            nc.vector.tensor_copy(kown[:], ps[:, 0:HT])
                            r = 128 * (2 * s + j)
                            nc.scalar.dma_start(k_dst[r:r + 128, :], kown[:])
                    # V for this half: token-major out
                    v_dst = kv_in[h][KV_H:2 * KV_H].rearrange(
                        "(t v) -> t v", v=INNER)
                    for s in range(INNER // WSL):
                        wt = wp.tile([128, DC * WSL], dt.bfloat16, tag="w")
                        nc.sync.dma_start(
                            wt[:],
                            wq[:, :, 2 * INNER + WSL * s:
                               2 * INNER + WSL * (s + 1)])
                        wt_v = wt.rearrange("p (c o) -> p c o", c=DC)
                        for t in range(2):
                            ps = pp.tile([128, TPC], dt.float32, tag="mm")
                            for c in range(DC):
                                nc.tensor.matmul(
                                    ps[:, 0:WSL],
                                    xln_v[:, c,
                                          HT * h + 128 * t:
                                          HT * h + 128 * (t + 1)],
                                    wt_v[:, c, :],
                                    start=(c == 0), stop=(c == DC - 1))
                            vown = tp3.tile([128, WSL], dt.bfloat16,
                                            tag="v_own")
                            nc.vector.tensor_copy(vown[:], ps[:, 0:WSL])
                            nc.scalar.dma_start(
                                v_dst[128 * t:128 * (t + 1),
                                      WSL * s:WSL * (s + 1)], vown[:])
                    nc.gpsimd.collective_compute(
                        "AllGather", OP.bypass,
                        ins=[kv_in[h].opt()], outs=[ag[h].opt()],
                        replica_groups=groups)

            for l in range(DEPTH):
                for h in range(2):
                    ln1_kv_half(l, h)
                kv_in, ag = kvag[l]
                wq = wqkv[l].rearrange("(c p) o -> p c o", p=128)
                dump("xln", xln[:], l)

                # ---- q for own tokens (overlaps the AllGathers)
                with nc.named_scope(f"L{l}_q"):
                    def q_consume(ps, jj):
                        nc.vector.tensor_copy(q_v[:, jj, :], ps[:])
                    stream_mm_dmajor(wq, 0, INNER, xln_v, q_consume)
                dump("q", qbf[:], l)

                # ---- load gathered k, v; key chunk ci = h*8 + r*2 + g
                with nc.named_scope(f"L{l}_kvload"):
                    for h in range(2):
                        for r in range(GROUP):
                            base = r * 2 * KV_H
                            off = (h * GROUP + r) * HT
                            k_src = ag[h][base:base + KV_H].rearrange(
                                "(c p t) -> p c t", p=128, t=HT)
                            nc.gpsimd.dma_start(
                                k_v[:, :, off:off + HT], k_src)
                            v_src = ag[h][base + KV_H:base + 2 * KV_H].rearrange(
                                "(g p hh e) -> p g hh e", p=128, hh=HEADS, e=DH)
                            for g in range(HT // 128):
                                nc.gpsimd.dma_start(
                                    vs_v[:, off // 128 + g, :, 0:DH],
                                    v_src[:, g])
                    if masked:
                        vv = vsb.rearrange("p (g w) -> p g w", g=KC)
                        for kc in range(KC):
                            nc.vector.tensor_scalar_mul(
                                vv[:, kc, :], vv[:, kc, :],
                                absb[:, kc:kc + 1])
                dump("ksb", ksb[:], l)
                dump("vsb", vsb[:], l)

                # ---- attention, head-pair at a time; the two 64-row score
                # matmuls of a pair run concurrently in disjoint PE row groups.
                with nc.named_scope(f"L{l}_attn"):
                    for hc in range(HEADS // 2):
                        h0, h1 = 2 * hc, 2 * hc + 1
                        po_a = ppo.tile([128, TPC], dt.float32, tag="attno")
                        po_b = ppo.tile([128, TPC], dt.float32, tag="attno")
                        es_q = [None] * KC

                        def attn_v(kc):
                            es2 = es_q[kc]
                            nc.tensor.matmul(
                                po_a[0:DH + 1, :],
                                vsb[:, kc * VW + h0 * (DH + 1):
                                    kc * VW + (h0 + 1) * (DH + 1)],
                                es2[:, 0:TPC],
                                start=(kc == 0), stop=(kc == KC - 1))
                            nc.tensor.matmul(
                                po_b[0:DH + 1, :],
                                vsb[:, kc * VW + h1 * (DH + 1):
                                    kc * VW + (h1 + 1) * (DH + 1)],
                                es2[:, TPC:2 * TPC],
                                start=(kc == 0), stop=(kc == KC - 1))

                        # skew-2 software pipeline: attnV(kc-2) issues after
                        # scores(kc) so the PE FIFO never waits on exp.
                        for kc in range(KC):
                            if kc >= 2:
                                attn_v(kc - 2)
                            ps2 = pp.tile([128, 2 * TPC], dt.float32, tag="mm")
                            nc.tensor.matmul(
                                ps2[:, 0:TPC],
                                k_v[0:DH, hc, 128 * kc:128 * (kc + 1)],
                                q_v[0:DH, hc, :],
                                start=True, stop=True, tile_position=(0, 0))
                            nc.tensor.matmul(
                                ps2[:, TPC:2 * TPC],
                                k_v[DH:128, hc, 128 * kc:128 * (kc + 1)],
                                q_v[DH:128, hc, :],
                                start=True, stop=True, tile_position=(64, 0))
                            es2 = esp.tile([128, 2 * TPC], dt.bfloat16,
                                           tag="es")
                            nc.scalar.activation(es2[:], ps2[:], AF.Exp,
                                                 scale=DH ** -0.5)
                            es_q[kc] = es2
                        attn_v(KC - 2)
                        attn_v(KC - 1)
                        # evacuate PSUM promptly (frees po for the next pair),
                        # then normalize from SBUF off the critical path.
                        for po, h in ((po_a, h0), (po_b, h1)):
                            hp = 64 * (h % 2)
                            pot = tp2.tile([DH + 1, TPC], dt.float32,
                                           tag="pot")
                            nc.vector.tensor_copy(pot[:], po[0:DH + 1, :])
                            # reciprocal on DVE: keeps the denominator chain
                            # off the exp-saturated ACT queue
                            rec = stp.tile([1, TPC], dt.float32, tag="rec")
                            nc.vector.reciprocal(rec[:], pot[DH:DH + 1, :])
                            bc = tp2.tile([64, TPC], dt.float32, tag="bc")
                            nc.gpsimd.partition_broadcast(bc[:], rec[:])
                            nc.vector.tensor_mul(
                                ot_v[hp:hp + DH, hc, :], pot[0:DH, :], bc[:])
                dump("ot", otsb[:], l)

                # ---- wo + residual
                with nc.named_scope(f"L{l}_wo"):
                    wov = wo[l].rearrange("(c p) o -> p c o", p=128)

                    def wo_consume(ps, jj):
                        nc.vector.tensor_add(x_v[:, jj, :], x_v[:, jj, :], ps[:])
                    stream_mm_dmajor(wov, 0, DIM, ot_v, wo_consume)
                dump("xattn", x_sb[:], l)

                with nc.named_scope(f"L{l}_ln2"):
                    layer_norm(2 * l + 1, x_v, xln_v)

                # ---- FFN
                with nc.named_scope(f"L{l}_ffn1"):
                    nc.sync.dma_start(
                        b1sb[:], b1[l].rearrange("(f p) -> p f", p=128))
                    w1v = w1[l].rearrange("(c p) o -> p c o", p=128)

                    def ffn1_consume(ps, f):
                        nc.scalar.activation(
                            h1_v[:, f, :], ps[:], AF.Gelu,
                            bias=b1sb[:, f:f + 1])
                    stream_mm_dmajor(w1v, 0, FF, xln_v, ffn1_consume)
                dump("h1", h1sb[:], l)

                nc.sync.dma_start(
                    b2sb[:], b2[l].rearrange("(c p) -> p c", p=128))
                w2v = w2[l].rearrange("(f p) o -> p f o", p=128)

                with nc.named_scope(f"L{l}_ffn2"):
                    for j in range(DC):
                        ps = ppo.tile([128, TPC], dt.float32, tag="attno")
                        for half in range(2):
                            wt = wp.tile([128, (FC // 2) * 128], dt.bfloat16,
                                         tag="w")
                            nc.sync.dma_start(
                                wt[:],
                                w2v[:, (FC // 2) * half:(FC // 2) * (half + 1),
                                    128 * j:128 * (j + 1)])
                            wt_v = wt.rearrange("p (f o) -> p f o", f=FC // 2)
                            for f2 in range(FC // 2):
                                f = (FC // 2) * half + f2
                                nc.tensor.matmul(
                                    ps[:], wt_v[:, f2, :], h1_v[:, f, :],
                                    start=(f == 0), stop=(f == FC - 1))
                        # x += psum + b2  (b2 is per-feature = per-partition)
                        t3 = tp2.tile([128, TPC], dt.float32, tag="res2")
                        nc.vector.tensor_scalar_add(
                            t3[:], ps[:], b2sb[:, j:j + 1])
                        nc.vector.tensor_add(x_v[:, j, :], x_v[:, j, :], t3[:])
                dump("xl0", x_sb[:], l)

            nc.sync.dma_start(xO.rearrange("(c p) t -> p c t", p=128),
                              x_v[:, :, :])

    nc.compile()
    return nc


def _prepare(inputs):
    x = np.ascontiguousarray(np.asarray(inputs["x"], dtype=np.float32))
    mask = np.asarray(inputs["mask"])
    wqkv = np.asarray(inputs["wqkv"], dtype=np.float32)
    wo = np.asarray(inputs["wo"], dtype=np.float32)
    w1 = np.asarray(inputs["w1"], dtype=np.float32)
    w2 = np.asarray(inputs["w2"], dtype=np.float32)
    b1 = np.asarray(inputs["b1"], dtype=np.float32)
    b2 = np.asarray(inputs["b2"], dtype=np.float32)
    ln1_g = np.asarray(inputs["ln1_g"], dtype=np.float32)
    ln1_b = np.asarray(inputs["ln1_b"], dtype=np.float32)
    ln2_g = np.asarray(inputs["ln2_g"], dtype=np.float32)
    ln2_b = np.asarray(inputs["ln2_b"], dtype=np.float32)

    ln_affine = not (
        np.all(ln1_g == 1.0) and np.all(ln1_b == 0.0)
        and np.all(ln2_g == 1.0) and np.all(ln2_b == 0.0))
    mask_b = np.asarray(mask, bool)
    masked = not np.all(mask_b)

    key = (ln_affine, masked)
    if key not in _CACHE:
        _CACHE[key] = _build(ln_affine, masked)
    nc = _CACHE[key]

    wqkv_bf = np.ascontiguousarray(wqkv.astype(BF16))
    wo_bf = np.ascontiguousarray(wo.astype(BF16))
    w1_bf = np.ascontiguousarray(w1.astype(BF16))
    w2_bf = np.ascontiguousarray(w2.astype(BF16))
    amult = mask_b.astype(np.float32)   # 1 visible / 0 masked

    in_maps = []
    for c in range(N_CORES):
        b = c // GROUP
        s = c % GROUP
        xT = np.ascontiguousarray(x[b, s * TPC:(s + 1) * TPC, :].T)
        m = {
            "xT": xT, "wqkv": wqkv_bf, "wo": wo_bf, "w1": w1_bf, "w2": w2_bf,
            "b1": b1, "b2": b2,
        }
        if masked:
            # permute keys to gathered chunk order: ci = h*8 + r*2 + g
            am = np.empty(SEQ, np.float32)
            ci = 0
            for h in range(2):
                for r in range(GROUP):
                    for g in range(HT // 128):
                        src = r * TPC + h * HT + g * 128
                        am[ci * 128:(ci + 1) * 128] = amult[b, src:src + 128]
                        ci += 1
            m["amult"] = np.ascontiguousarray(am)
        if ln_affine:
            g = np.stack([v for pair in zip(ln1_g, ln2_g) for v in pair])
            bb = np.stack([v for pair in zip(ln1_b, ln2_b) for v in pair])
            m["ln_g"] = np.ascontiguousarray(g)
            m["ln_b"] = np.ascontiguousarray(bb)
        in_maps.append(m)
    return nc, in_maps


def _assemble(results):
    out = np.empty((BATCH, SEQ, DIM), np.float32)
    for c in range(N_CORES):
        b = c // GROUP
        s = c % GROUP
        out[b, s * TPC:(s + 1) * TPC, :] = results[c]["xO"].T
    return out


def kernel(**inputs):
    nc, in_maps = _prepare(inputs)
    res = run_bass_kernel_spmd(nc, in_maps, list(range(N_CORES)))
    return _assemble(res.results)


def run_traced(**inputs):
    """Used by test.py only: run with NTFF profiling enabled."""
    nc, in_maps = _prepare(inputs)
    return run_bass_kernel_spmd(nc, in_maps, list(range(N_CORES)), trace=True)
